# revision 15
# baseline (speedup 1.0000x reference)
"""Causal self-attention (B=4, T=2048, C=1024, H=16) on 8 TRN2 NeuronCores.

Sharding: core = 2*b + hg  (b = batch 0..3, hg = head-group 0..1, 8 heads each).
Datapath is bf16 matmuls with fp32 PSUM/softmax/normalization:
  prologue: k^T and v (with an interleaved ones-column per head for softmax
            denominators) for all 16 key chunks; x stays resident in SBUF.
  main loop over query chunks n: scores^T = k^T.T @ q^T per head pair
            (row-groups 0/64 pack the two heads), exp on ACT straight from
            PSUM, causal triangle via gpsimd affine_select, AV matmul with
            appended ones column, deferred per-query normalization; the q^T
            projection for chunk n+1 and the output projection for chunk n-1
            are interleaved into the same loop to keep the PE array dense.
  output:   after each head-pair norm the y piece [128,512] is AllGathered
            across the batch pair; each core then computes its own half of
            the output CHANNELS (full 1024-row contraction, permuted W_proj
            prepared host-side) and writes straight to out^T in DRAM. No
            ReduceScatter; only the final 128KB AllGather is on the
            critical path.
Host side transposes x per batch on the way in and reassembles/transposes the
output on the way out.
"""
import numpy as np
import ml_dtypes
from contextlib import ExitStack

import concourse.bass as bass
from concourse import bacc, mybir
from concourse.tile import TileContext
from concourse.bass_utils import run_bass_kernel_spmd

dt = mybir.dt
AF = mybir.ActivationFunctionType
BF16 = ml_dtypes.bfloat16

B, T, C, H = 4, 2048, 1024, 16
D = 64              # head dim
HL = 8              # heads per core
CL = HL * D         # 512 local channels
NQ = T // 512       # 4 query chunks of 512
NT = T // 128       # 16 key/time chunks of 128
SCALE = 1.0 / np.sqrt(D)

_CACHE = {}


def _build_nc():
    nc = bacc.Bacc("TRN2", target_bir_lowering=False, debug=False)

    xT_e = nc.declare_dram_parameter("xT", [C, T], dt.bfloat16, isOutput=False)
    wkv_e = nc.declare_dram_parameter("wkv", [C, 2 * CL], dt.bfloat16, isOutput=False)
    wq_e = nc.declare_dram_parameter("wq", [C, CL], dt.bfloat16, isOutput=False)
    wp_e = nc.declare_dram_parameter("wperm", [C, CL], dt.bfloat16, isOutput=False)
    bqk_e = nc.declare_dram_parameter("bqk", [128, 8], dt.float32, isOutput=False)
    bvr_e = nc.declare_dram_parameter("bvr", [1, CL], dt.bfloat16, isOutput=False)
    bp_e = nc.declare_dram_parameter("bproj", [128, 4], dt.float32, isOutput=False)
    out_e = nc.declare_dram_parameter("out", [CL, T], dt.float32, isOutput=True)

    RG = [[0, 1], [2, 3], [4, 5], [6, 7]]

    with TileContext(nc) as tc, nc.allow_low_precision("bf16 datapath by design"):
        with ExitStack() as top:
            p_cst = top.enter_context(tc.tile_pool(name="cst", bufs=1))
            p_x = top.enter_context(tc.tile_pool(name="xres", bufs=8))
            p_kt = top.enter_context(tc.tile_pool(name="kt", bufs=4))
            p_v = top.enter_context(tc.tile_pool(name="v", bufs=16))
            p_wq = top.enter_context(tc.tile_pool(name="wq", bufs=8))
            p_wp = top.enter_context(tc.tile_pool(name="wp", bufs=8))
            pp_wk = top.enter_context(tc.tile_pool(name="ppwk", bufs=2, space="PSUM"))
            pp_q = top.enter_context(tc.tile_pool(name="ppq", bufs=1, space="PSUM"))

            ones_f = p_cst.tile([128, 128], dt.float32)
            nc.gpsimd.memset(ones_f[:], 1.0)
            ones_bf = p_cst.tile([1, 128], dt.bfloat16)
            nc.vector.tensor_copy(ones_bf[:], ones_f[0:1, :])
            bqk_sb = p_cst.tile([128, 8], dt.float32)
            bp_sb = p_cst.tile([128, 4], dt.float32)
            bvr_sb = p_cst.tile([1, CL], dt.bfloat16)

            x_sb = [p_x.tile([128, T], dt.bfloat16, tag="x", name=f"x{c}")
                    for c in range(8)]
            kt_sb = [p_kt.tile([128, T], dt.bfloat16, tag="kt", name=f"ktt{i}")
                     for i in range(4)]
            v_sb = [p_v.tile([128, 8 * 65], dt.bfloat16, tag="v", name=f"vt{i}")
                    for i in range(NT)]

            # ---------------- prologue: k^T and v for all chunks ----------------
            with ExitStack() as pctx:
                p_wkv = pctx.enter_context(tc.tile_pool(name="wkv", bufs=8))
                # first matmul group's operands first (k-half weights + x cols
                # 0:512, pairwise) so PE starts ~immediately; v-half weights
                # and the rest of x stream behind.
                # descriptor-gen is ~0.6us per DMA on a queue: issue x on the
                # scalar queue and weights on sync concurrently, big DMAs only
                wkv_sb = []
                for c in range(8):
                    wt = p_wkv.tile([128, 1024], dt.bfloat16, tag="wkv", name=f"wkvt{c}")
                    nc.sync.dma_start(wt[:], wkv_e[c * 128:(c + 1) * 128, :])
                    wkv_sb.append(wt)
                    nc.scalar.dma_start(x_sb[c][:, 0:512], xT_e[c * 128:(c + 1) * 128, 0:512])
                nc.sync.dma_start(bqk_sb[:], bqk_e[:])
                nc.sync.dma_start(bp_sb[:], bp_e[:])
                nc.sync.dma_start(bvr_sb[:], bvr_e[:])
                for c in range(8):
                    nc.scalar.dma_start(x_sb[c][:, 512:2048],
                                        xT_e[c * 128:(c + 1) * 128, 512:2048])

                for n in range(NQ):
                    xof = n * 512
                    for mk in range(4):
                        ps_t = pp_wk.tile([128, 1024], dt.float32, tag="wk")
                        for c in range(8):
                            nc.tensor.matmul(ps_t[:, 0:512], wkv_sb[c][:, mk * 128:(mk + 1) * 128],
                                             x_sb[c][:, xof:xof + 512],
                                             start=(c == 0), stop=(c == 7))
                        nc.scalar.activation(kt_sb[mk][:, xof:xof + 512], ps_t[:, 0:512],
                                             AF.Identity, bias=bqk_sb[:, 4 + mk:5 + mk])
                    for tv in range(4):
                        ps_v = pp_q.tile([128, 512], dt.float32, tag="qv")
                        for c in range(8):
                            nc.tensor.matmul(ps_v[:], x_sb[c][:, xof + tv * 128:xof + (tv + 1) * 128],
                                             wkv_sb[c][:, 512:1024], start=(c == 0), stop=False)
                        nc.tensor.matmul(ps_v[:], ones_bf[:], bvr_sb[:], start=False, stop=True)
                        vt = v_sb[n * 4 + tv]
                        nc.scalar.activation(
                            vt[:].rearrange("p (h s) -> p h s", s=65)[:, :, 0:64],
                            ps_v[:].rearrange("p (h s) -> p h s", s=64),
                            AF.Copy)
                        nc.vector.tensor_copy(vt[:, 64:520:65], ones_f[:, 0:8])

            # ---------------- main loop ----------------
            wq_sb = []
            for c in range(8):
                wqt = p_wq.tile([128, CL], dt.bfloat16, tag="wq", name=f"wqt{c}")
                nc.sync.dma_start(wqt[:], wq_e[c * 128:(c + 1) * 128, :])
                wq_sb.append(wqt)
            wp_sb = []
            for p in range(8):
                wpt = p_wp.tile([128, CL], dt.bfloat16, tag="wp", name=f"wpt{p}")
                nc.sync.dma_start(wpt[:], wp_e[p * 128:(p + 1) * 128, :])
                wp_sb.append(wpt)

            with ExitStack() as bctx:
                p_q = bctx.enter_context(tc.tile_pool(name="q", bufs=8))
                p_att = bctx.enter_context(tc.tile_pool(name="att", bufs=5))
                p_y = bctx.enter_context(tc.tile_pool(name="yt", bufs=4))
                p_yg = bctx.enter_context(tc.tile_pool(name="yg", bufs=16))
                p_rec = bctx.enter_context(tc.tile_pool(name="rec", bufs=2))
                p_bc = bctx.enter_context(tc.tile_pool(name="bc", bufs=2))
                p_out = bctx.enter_context(tc.tile_pool(name="osb", bufs=4))
                pp_y = bctx.enter_context(tc.tile_pool(name="ppy", bufs=3, space="PSUM"))
                p_dram = bctx.enter_context(tc.tile_pool(name="agd", bufs=4, space="DRAM"))

                q_tiles = {}      # n -> [4 tiles of [128, 512]]
                yg_tiles = {}     # n -> [8 gathered y pieces [128, 512]]
                pair_store = {}   # (n, hp, j) -> (m0, m1, {h: (a_t, q0, q1)})
                ypss_store = {}   # (n, hp) -> {h: y_ps}

                def emit_q_slice(n, mq):
                    if mq == 0:
                        q_tiles[n] = []
                    ps_t = pp_q.tile([128, 512], dt.float32, tag="qv")
                    for c in range(8):
                        nc.tensor.matmul(ps_t[:], wq_sb[c][:, mq * 128:(mq + 1) * 128],
                                         x_sb[c][:, n * 512:(n + 1) * 512],
                                         start=(c == 0), stop=(c == 7))
                    qt = p_q.tile([128, 512], dt.bfloat16, tag="q", name=f"q{n}_{mq}")
                    nc.vector.tensor_scalar_add(qt[:], ps_t[:], bqk_sb[:, mq:mq + 1])
                    q_tiles[n].append(qt)

                def emit_scores_pair(n, hp, j):
                    h0, h1 = 2 * hp, 2 * hp + 1
                    if j == 0:
                        ypss_store[(n, hp)] = {
                            h: pp_y.tile([128, 512], dt.float32, tag="ypsum",
                                         name=f"yps{n}_{h}")
                            for h in (h0, h1)}
                    m0, m1 = 2 * j, 2 * j + 1
                    r0, r1 = m0 - 4 * n, m1 - 4 * n
                    q0 = 128 * r0 if r0 >= 0 else 0
                    q1 = 128 * r1 if r1 >= 0 else 0
                    entry = {}
                    for h in (h0, h1):
                        base = (h % 2) * 64
                        qt = q_tiles[n][h // 2]
                        kt = kt_sb[h // 2]
                        s_ps = pp_wk.tile([128, 1024], dt.float32, tag="wk")
                        nc.tensor.matmul(
                            s_ps[:, q0:512],
                            kt[base:base + 64, m0 * 128:(m0 + 1) * 128],
                            qt[base:base + 64, q0:512],
                            start=True, stop=True)
                        nc.tensor.matmul(
                            s_ps[:, 512 + q1:1024],
                            kt[base:base + 64, m1 * 128:(m1 + 1) * 128],
                            qt[base:base + 64, q1:512],
                            start=True, stop=True)
                        a_t = p_att.tile([128, 1024], dt.bfloat16, tag="att",
                                         name=f"a{n}_{hp}_{j}_{h}")
                        nc.scalar.activation(a_t[:, q0:1024], s_ps[:, q0:1024],
                                             AF.Exp, scale=float(SCALE))
                        if r0 >= 0:
                            nc.gpsimd.affine_select(
                                out=a_t[:, q0:q0 + 128], in_=a_t[:, q0:q0 + 128],
                                compare_op=mybir.AluOpType.is_ge, fill=0.0, base=0,
                                pattern=[[1, 128]], channel_multiplier=-1)
                        if r1 >= 0:
                            nc.gpsimd.affine_select(
                                out=a_t[:, 512 + q1:512 + q1 + 128],
                                in_=a_t[:, 512 + q1:512 + q1 + 128],
                                compare_op=mybir.AluOpType.is_ge, fill=0.0, base=0,
                                pattern=[[1, 128]], channel_multiplier=-1)
                        entry[h] = (a_t, q0, q1)
                    pair_store[(n, hp, j)] = (m0, m1, entry)

                def emit_avs_pair(n, hp, j):
                    m_max = 4 * n + 4
                    h0, h1 = 2 * hp, 2 * hp + 1
                    y_pss = ypss_store[(n, hp)]
                    m0, m1, entry = pair_store.pop((n, hp, j))
                    for h in (h0, h1):
                        a_t, q0, q1 = entry[h]
                        nc.tensor.matmul(
                            y_pss[h][0:65, q0:512],
                            v_sb[m0][:, h * 65:h * 65 + 65],
                            a_t[:, q0:512],
                            start=(m0 == 0), stop=False)
                        nc.tensor.matmul(
                            y_pss[h][0:65, q1:512],
                            v_sb[m1][:, h * 65:h * 65 + 65],
                            a_t[:, 512 + q1:1024],
                            start=False, stop=(m1 == m_max - 1))

                def emit_norm(n, hp):
                    h0, h1 = 2 * hp, 2 * hp + 1
                    y_pss = ypss_store.pop((n, hp))
                    yt = p_y.tile([128, 512], dt.bfloat16, tag="yt", name=f"yt{n}_{hp}")
                    rec_s = p_rec.tile([128, 1024], dt.float32, tag="recs")
                    rec = p_rec.tile([128, 1024], dt.float32, tag="rec")
                    nc.vector.tensor_copy(rec_s[0:1, 0:512], y_pss[h0][64:65, :])
                    nc.vector.tensor_copy(rec_s[0:1, 512:1024], y_pss[h1][64:65, :])
                    nc.vector.reciprocal_approx_fast(out=rec[0:1, :], in_=rec_s[0:1, :])
                    bc_sb = p_bc.tile([128, 1024], dt.float32)
                    nc.gpsimd.partition_broadcast(bc_sb[0:64, :], rec[0:1, :],
                                                  channels=64)
                    for h in (h0, h1):
                        base = (h % 2) * 64
                        half = (h % 2) * 512
                        nc.vector.tensor_mul(yt[base:base + 64, :], y_pss[h][0:64, :],
                                             bc_sb[0:64, half:half + 512])
                    # exchange this y piece with the pair partner right away
                    ag_in = p_dram.tile([128, 512], dt.bfloat16, tag="agi",
                                        name=f"agi{n}_{hp}")
                    ag_out = p_dram.tile([256, 512], dt.bfloat16, tag="ago",
                                         name=f"ago{n}_{hp}")
                    nc.sync.dma_start(ag_in[:], yt[:])
                    nc.gpsimd.collective_compute(
                        "AllGather", mybir.AluOpType.bypass,
                        ins=[ag_in[:]], outs=[ag_out[:]], replica_groups=RG)
                    for half in range(2):
                        g = p_yg.tile([128, 512], dt.bfloat16, tag="yg",
                                      name=f"yg{n}_{2 * hp + half}")
                        nc.sync.dma_start(g[:], ag_out[half * 128:(half + 1) * 128, :])
                        yg_tiles.setdefault(n, []).append(g)

                def emit_c_chunk(n, co):
                    o_ps = pp_wk.tile([128, 1024], dt.float32, tag="wk")
                    for p in range(8):
                        nc.tensor.matmul(o_ps[:, 0:512], wp_sb[p][:, co * 128:(co + 1) * 128],
                                         yg_tiles[n][p][:], start=(p == 0), stop=(p == 7))
                    o_sb = p_out.tile([128, 512], dt.float32)
                    nc.vector.tensor_scalar_add(o_sb[:], o_ps[:, 0:512], bp_sb[:, co:co + 1])
                    nc.sync.dma_start(out_e[co * 128:(co + 1) * 128, n * 512:(n + 1) * 512],
                                      o_sb[:])

                def emit_filler(f):
                    if f[0] == "q":
                        emit_q_slice(f[1], f[2])
                    else:
                        emit_c_chunk(f[1], f[2])

                for step in range(5):
                    bn = step - 1
                    qn = step if step < NQ else -1
                    cn = step - 2
                    fillers = []
                    if qn >= 0:
                        fillers += [("q", qn, mq) for mq in range(4)]
                    if cn >= 0:
                        # on the last step hold back two projection chunks to
                        # fill the PE while the final AllGather is in flight
                        ncos = 2 if step == 4 else 4
                        fillers += [("c", cn, co) for co in range(ncos)]
                    if bn < 0:
                        for f in fillers:
                            emit_filler(f)
                        continue
                    pairs_total = (2 * bn + 2) * 4
                    k = 0
                    fi = 0
                    for hp in range(4):
                        npair = 2 * bn + 2
                        for j in range(npair):
                            emit_scores_pair(bn, hp, j)
                            while fi < len(fillers) and fi * pairs_total < (k + 1) * len(fillers):
                                emit_filler(fillers[fi])
                                fi += 1
                            if j >= 1:
                                emit_avs_pair(bn, hp, j - 1)
                            k += 1
                        emit_avs_pair(bn, hp, npair - 1)
                        emit_norm(bn, hp)
                    while fi < len(fillers):
                        emit_filler(fillers[fi])
                        fi += 1

                # epilogue: the held-back chunk-2 projections (independent of
                # the final AllGather) fill the PE while it is in flight, then
                # chunk 3's projection. Pieces 0..5 emit piece-major; the last
                # two pieces run co-major so each co's bias-add + store
                # pipelines as soon as its group stops.
                emit_c_chunk(2, 2)
                emit_c_chunk(2, 3)
                n = 3
                opsA = pp_wk.tile([128, 1024], dt.float32, tag="wk")
                opsB = pp_wk.tile([128, 1024], dt.float32, tag="wk")
                regions = [opsA[:, 0:512], opsA[:, 512:1024],
                           opsB[:, 0:512], opsB[:, 512:1024]]
                for p in range(6):
                    for co in range(4):
                        nc.tensor.matmul(regions[co], wp_sb[p][:, co * 128:(co + 1) * 128],
                                         yg_tiles[n][p][:], start=(p == 0), stop=False)
                for co in range(4):
                    for p in (6, 7):
                        nc.tensor.matmul(regions[co], wp_sb[p][:, co * 128:(co + 1) * 128],
                                         yg_tiles[n][p][:], start=False, stop=(p == 7))
                    o_sb = p_out.tile([128, 512], dt.float32)
                    nc.vector.tensor_scalar_add(o_sb[:], regions[co], bp_sb[:, co:co + 1])
                    nc.sync.dma_start(out_e[co * 128:(co + 1) * 128, n * 512:(n + 1) * 512],
                                      o_sb[:])

    nc.finalize()
    return nc


def _get_nc():
    if "nc" not in _CACHE:
        _CACHE["nc"] = _build_nc()
    return _CACHE["nc"]


def _make_in_maps(x, W_attn, b_attn, W_proj, b_proj):
    x = np.asarray(x, dtype=np.float32)
    W_attn = np.asarray(W_attn, dtype=np.float32)
    b_attn = np.asarray(b_attn, dtype=np.float32)
    W_proj = np.asarray(W_proj, dtype=np.float32)
    b_proj = np.asarray(b_proj, dtype=np.float32)

    in_maps = []
    for core in range(8):
        b, hg = core // 2, core % 2
        lo, hi = hg * CL, (hg + 1) * CL
        wq = W_attn[:, lo:hi]
        wk = W_attn[:, C + lo:C + hi]
        wv = W_attn[:, 2 * C + lo:2 * C + hi]
        bq = b_attn[lo:hi]
        bk = b_attn[C + lo:C + hi]
        bv = b_attn[2 * C + lo:2 * C + hi]
        # permuted W_proj rows: per hp, the even core's two heads then the
        # odd core's two heads (matches AllGather piece arrival order);
        # columns are this core's half of the output channels.
        wp_rows = [W_proj[128 * hp + 512 * par:128 * hp + 512 * par + 128, lo:hi]
                   for hp in range(4) for par in range(2)]
        in_maps.append({
            "xT": np.ascontiguousarray(x[b].T.astype(BF16)),
            "wkv": np.ascontiguousarray(
                np.concatenate([wk, wv], axis=1).astype(BF16)),
            "wq": np.ascontiguousarray(wq.astype(BF16)),
            "wperm": np.ascontiguousarray(
                np.concatenate(wp_rows, axis=0).astype(BF16)),
            "bqk": np.ascontiguousarray(np.concatenate([bq, bk]).reshape(8, 128).T),
            "bvr": np.ascontiguousarray(bv.reshape(1, CL).astype(BF16)),
            "bproj": np.ascontiguousarray(b_proj[lo:hi].reshape(4, 128).T),
        })
    return in_maps


def _assemble(results):
    out = np.empty((B, T, C), dtype=np.float32)
    outT = np.empty((C, T), dtype=np.float32)
    for b in range(B):
        outT[0:512, :] = results[2 * b]["out"]
        outT[512:1024, :] = results[2 * b + 1]["out"]
        out[b] = outT.T
    return out


def run(trace=False, **inputs):
    nc = _get_nc()
    in_maps = _make_in_maps(**inputs)
    kw = {}
    if trace:
        kw = dict(trace=True, trace_cores=[0])
    res = run_bass_kernel_spmd(nc, in_maps, list(range(8)), **kw)
    return _assemble(res.results), res


def kernel(**inputs) -> np.ndarray:
    out, _ = run(trace=False, **inputs)
    return out


# revision 19
# speedup vs baseline: 1.0370x; 1.0370x over previous
"""Causal self-attention (B=4, T=2048, C=1024, H=16) on 8 TRN2 NeuronCores.

Sharding: core = 2*b + hg  (b = batch 0..3, hg = head-group 0..1, 8 heads each).
Datapath is bf16 matmuls with fp32 PSUM/softmax/normalization:
  prologue: k^T and v (with an interleaved ones-column per head for softmax
            denominators) for all 16 key chunks; x stays resident in SBUF.
  main loop over query chunks n: scores^T = k^T.T @ q^T per head pair
            (row-groups 0/64 pack the two heads), exp on ACT straight from
            PSUM, causal triangle via gpsimd affine_select, AV matmul with
            appended ones column, deferred per-query normalization; the q^T
            projection for chunk n+1 and the output projection for chunk n-1
            are interleaved into the same loop to keep the PE array dense.
  output:   after each head-pair norm the y piece [128,512] is AllGathered
            across the batch pair; each core then computes its own half of
            the output CHANNELS (full 1024-row contraction, permuted W_proj
            prepared host-side) and writes straight to out^T in DRAM. No
            ReduceScatter; only the final 128KB AllGather is on the
            critical path.
Host side transposes x per batch on the way in and reassembles/transposes the
output on the way out.
"""
import numpy as np
import ml_dtypes
from contextlib import ExitStack

import concourse.bass as bass
from concourse import bacc, mybir
from concourse.tile import TileContext
from concourse.bass_utils import run_bass_kernel_spmd

dt = mybir.dt
AF = mybir.ActivationFunctionType
BF16 = ml_dtypes.bfloat16

B, T, C, H = 4, 2048, 1024, 16
D = 64              # head dim
HL = 8              # heads per core
CL = HL * D         # 512 local channels
NQ = T // 512       # 4 query chunks of 512
NT = T // 128       # 16 key/time chunks of 128
SCALE = 1.0 / np.sqrt(D)

_CACHE = {}


def _build_nc():
    nc = bacc.Bacc("TRN2", target_bir_lowering=False, debug=False)

    xT_e = nc.declare_dram_parameter("xT", [C, T], dt.bfloat16, isOutput=False)
    wkv_e = nc.declare_dram_parameter("wkv", [C, 2 * CL], dt.bfloat16, isOutput=False)
    wq_e = nc.declare_dram_parameter("wq", [C, CL], dt.bfloat16, isOutput=False)
    wp_e = nc.declare_dram_parameter("wperm", [C, CL], dt.bfloat16, isOutput=False)
    bqk_e = nc.declare_dram_parameter("bqk", [128, 8], dt.float32, isOutput=False)
    bvr_e = nc.declare_dram_parameter("bvr", [1, CL], dt.bfloat16, isOutput=False)
    bp_e = nc.declare_dram_parameter("bproj", [128, 4], dt.float32, isOutput=False)
    out_e = nc.declare_dram_parameter("out", [CL, T], dt.float32, isOutput=True)

    RG = [[0, 1], [2, 3], [4, 5], [6, 7]]

    with TileContext(nc) as tc, nc.allow_low_precision("bf16 datapath by design"):
        with ExitStack() as top:
            p_cst = top.enter_context(tc.tile_pool(name="cst", bufs=1))
            p_x = top.enter_context(tc.tile_pool(name="xres", bufs=8))
            p_kt = top.enter_context(tc.tile_pool(name="kt", bufs=4))
            p_v = top.enter_context(tc.tile_pool(name="v", bufs=16))
            p_wq = top.enter_context(tc.tile_pool(name="wq", bufs=8))
            p_wp = top.enter_context(tc.tile_pool(name="wp", bufs=8))
            pp_wk = top.enter_context(tc.tile_pool(name="ppwk", bufs=2, space="PSUM"))
            pp_q = top.enter_context(tc.tile_pool(name="ppq", bufs=1, space="PSUM"))

            ones_f = p_cst.tile([128, 128], dt.float32)
            nc.gpsimd.memset(ones_f[:], 1.0)
            ones_bf = p_cst.tile([1, 128], dt.bfloat16)
            nc.vector.tensor_copy(ones_bf[:], ones_f[0:1, :])
            bqk_sb = p_cst.tile([128, 8], dt.float32)
            bp_sb = p_cst.tile([128, 4], dt.float32)
            bvr_sb = p_cst.tile([1, CL], dt.bfloat16)

            x_sb = [p_x.tile([128, T], dt.bfloat16, tag="x", name=f"x{c}")
                    for c in range(8)]
            kt_sb = [p_kt.tile([128, T], dt.bfloat16, tag="kt", name=f"ktt{i}")
                     for i in range(4)]
            v_sb = [p_v.tile([128, 8 * 65], dt.bfloat16, tag="v", name=f"vt{i}")
                    for i in range(NT)]

            # ---------------- prologue: k^T and v for all chunks ----------------
            with ExitStack() as pctx:
                p_wkv = pctx.enter_context(tc.tile_pool(name="wkv", bufs=8))
                # first matmul group's operands first (k-half weights + x cols
                # 0:512, pairwise) so PE starts ~immediately; v-half weights
                # and the rest of x stream behind.
                # descriptor-gen is ~0.6us per DMA on a queue: issue x on the
                # scalar queue and weights on sync concurrently, big DMAs only
                wkv_sb = []
                for c in range(8):
                    wt = p_wkv.tile([128, 1024], dt.bfloat16, tag="wkv", name=f"wkvt{c}")
                    nc.sync.dma_start(wt[:], wkv_e[c * 128:(c + 1) * 128, :])
                    wkv_sb.append(wt)
                    nc.scalar.dma_start(x_sb[c][:, 0:512], xT_e[c * 128:(c + 1) * 128, 0:512])
                nc.scalar.dma_start(bqk_sb[:], bqk_e[:])
                nc.scalar.dma_start(bp_sb[:], bp_e[:])
                nc.scalar.dma_start(bvr_sb[:], bvr_e[:])
                for c in range(8):
                    nc.sync.dma_start(x_sb[c][:, 512:2048],
                                      xT_e[c * 128:(c + 1) * 128, 512:2048])

                for n in range(NQ):
                    xof = n * 512
                    for mk in range(4):
                        ps_t = pp_wk.tile([128, 1024], dt.float32, tag="wk")
                        for c in range(8):
                            nc.tensor.matmul(ps_t[:, 0:512], wkv_sb[c][:, mk * 128:(mk + 1) * 128],
                                             x_sb[c][:, xof:xof + 512],
                                             start=(c == 0), stop=(c == 7))
                        nc.scalar.activation(kt_sb[mk][:, xof:xof + 512], ps_t[:, 0:512],
                                             AF.Identity, bias=bqk_sb[:, 4 + mk:5 + mk])
                    for tv in range(4):
                        ps_v = pp_q.tile([128, 512], dt.float32, tag="qv")
                        for c in range(8):
                            nc.tensor.matmul(ps_v[:], x_sb[c][:, xof + tv * 128:xof + (tv + 1) * 128],
                                             wkv_sb[c][:, 512:1024], start=(c == 0), stop=False)
                        nc.tensor.matmul(ps_v[:], ones_bf[:], bvr_sb[:], start=False, stop=True)
                        vt = v_sb[n * 4 + tv]
                        nc.scalar.activation(
                            vt[:].rearrange("p (h s) -> p h s", s=65)[:, :, 0:64],
                            ps_v[:].rearrange("p (h s) -> p h s", s=64),
                            AF.Copy)
                        nc.vector.tensor_copy(vt[:, 64:520:65], ones_f[:, 0:8])

            # ---------------- main loop ----------------
            wq_sb = []
            for c in range(8):
                wqt = p_wq.tile([128, CL], dt.bfloat16, tag="wq", name=f"wqt{c}")
                nc.sync.dma_start(wqt[:], wq_e[c * 128:(c + 1) * 128, :])
                wq_sb.append(wqt)
            wp_sb = []
            for p in range(8):
                wpt = p_wp.tile([128, CL], dt.bfloat16, tag="wp", name=f"wpt{p}")
                nc.sync.dma_start(wpt[:], wp_e[p * 128:(p + 1) * 128, :])
                wp_sb.append(wpt)

            with ExitStack() as bctx:
                p_q = bctx.enter_context(tc.tile_pool(name="q", bufs=8))
                p_att = bctx.enter_context(tc.tile_pool(name="att", bufs=5))
                p_y = bctx.enter_context(tc.tile_pool(name="yt", bufs=4))
                p_yg = bctx.enter_context(tc.tile_pool(name="yg", bufs=16))
                p_rec = bctx.enter_context(tc.tile_pool(name="rec", bufs=2))
                p_bc = bctx.enter_context(tc.tile_pool(name="bc", bufs=2))
                p_out = bctx.enter_context(tc.tile_pool(name="osb", bufs=4))
                pp_y = bctx.enter_context(tc.tile_pool(name="ppy", bufs=3, space="PSUM"))
                p_dram = bctx.enter_context(tc.tile_pool(name="agd", bufs=4, space="DRAM"))

                q_tiles = {}      # n -> [4 tiles of [128, 512]]
                yg_tiles = {}     # n -> [8 gathered y pieces [128, 512]]
                pair_store = {}   # (n, hp, j) -> (m0, m1, {h: (a_t, q0, q1)})
                ypss_store = {}   # (n, hp) -> {h: y_ps}

                def emit_q_slice(n, mq):
                    if mq == 0:
                        q_tiles[n] = []
                    ps_t = pp_q.tile([128, 512], dt.float32, tag="qv")
                    for c in range(8):
                        nc.tensor.matmul(ps_t[:], wq_sb[c][:, mq * 128:(mq + 1) * 128],
                                         x_sb[c][:, n * 512:(n + 1) * 512],
                                         start=(c == 0), stop=(c == 7))
                    qt = p_q.tile([128, 512], dt.bfloat16, tag="q", name=f"q{n}_{mq}")
                    nc.vector.tensor_scalar_add(qt[:], ps_t[:], bqk_sb[:, mq:mq + 1])
                    q_tiles[n].append(qt)

                def emit_scores_pair(n, hp, j):
                    h0, h1 = 2 * hp, 2 * hp + 1
                    if j == 0:
                        ypss_store[(n, hp)] = {
                            h: pp_y.tile([128, 512], dt.float32, tag="ypsum",
                                         name=f"yps{n}_{h}")
                            for h in (h0, h1)}
                    m0, m1 = 2 * j, 2 * j + 1
                    r0, r1 = m0 - 4 * n, m1 - 4 * n
                    q0 = 128 * r0 if r0 >= 0 else 0
                    q1 = 128 * r1 if r1 >= 0 else 0
                    entry = {}
                    for h in (h0, h1):
                        base = (h % 2) * 64
                        qt = q_tiles[n][h // 2]
                        kt = kt_sb[h // 2]
                        s_ps = pp_wk.tile([128, 1024], dt.float32, tag="wk")
                        nc.tensor.matmul(
                            s_ps[:, q0:512],
                            kt[base:base + 64, m0 * 128:(m0 + 1) * 128],
                            qt[base:base + 64, q0:512],
                            start=True, stop=True)
                        nc.tensor.matmul(
                            s_ps[:, 512 + q1:1024],
                            kt[base:base + 64, m1 * 128:(m1 + 1) * 128],
                            qt[base:base + 64, q1:512],
                            start=True, stop=True)
                        a_t = p_att.tile([128, 1024], dt.bfloat16, tag="att",
                                         name=f"a{n}_{hp}_{j}_{h}")
                        nc.scalar.activation(a_t[:, q0:1024], s_ps[:, q0:1024],
                                             AF.Exp, scale=float(SCALE))
                        if r0 >= 0:
                            nc.gpsimd.affine_select(
                                out=a_t[:, q0:q0 + 128], in_=a_t[:, q0:q0 + 128],
                                compare_op=mybir.AluOpType.is_ge, fill=0.0, base=0,
                                pattern=[[1, 128]], channel_multiplier=-1)
                        if r1 >= 0:
                            nc.gpsimd.affine_select(
                                out=a_t[:, 512 + q1:512 + q1 + 128],
                                in_=a_t[:, 512 + q1:512 + q1 + 128],
                                compare_op=mybir.AluOpType.is_ge, fill=0.0, base=0,
                                pattern=[[1, 128]], channel_multiplier=-1)
                        entry[h] = (a_t, q0, q1)
                    pair_store[(n, hp, j)] = (m0, m1, entry)

                def emit_avs_pair(n, hp, j):
                    m_max = 4 * n + 4
                    h0, h1 = 2 * hp, 2 * hp + 1
                    y_pss = ypss_store[(n, hp)]
                    m0, m1, entry = pair_store.pop((n, hp, j))
                    for h in (h0, h1):
                        a_t, q0, q1 = entry[h]
                        nc.tensor.matmul(
                            y_pss[h][0:65, q0:512],
                            v_sb[m0][:, h * 65:h * 65 + 65],
                            a_t[:, q0:512],
                            start=(m0 == 0), stop=False)
                        nc.tensor.matmul(
                            y_pss[h][0:65, q1:512],
                            v_sb[m1][:, h * 65:h * 65 + 65],
                            a_t[:, 512 + q1:1024],
                            start=False, stop=(m1 == m_max - 1))

                def emit_norm(n, hp):
                    h0, h1 = 2 * hp, 2 * hp + 1
                    y_pss = ypss_store.pop((n, hp))
                    yt = p_y.tile([128, 512], dt.bfloat16, tag="yt", name=f"yt{n}_{hp}")
                    ag_in = p_dram.tile([128, 512], dt.bfloat16, tag="agi",
                                        name=f"agi{n}_{hp}")
                    ag_out = p_dram.tile([256, 512], dt.bfloat16, tag="ago",
                                         name=f"ago{n}_{hp}")
                    for h in (h0, h1):
                        base = (h % 2) * 64
                        rec_s = p_rec.tile([128, 512], dt.float32, tag="recs")
                        rec = p_rec.tile([128, 512], dt.float32, tag="rec")
                        nc.vector.tensor_copy(rec_s[0:1, :], y_pss[h][64:65, :])
                        nc.vector.reciprocal_approx_fast(out=rec[0:1, :], in_=rec_s[0:1, :])
                        bc_sb = p_bc.tile([128, 512], dt.float32)
                        nc.gpsimd.partition_broadcast(bc_sb[0:64, :], rec[0:1, :],
                                                      channels=64)
                        nc.vector.tensor_mul(yt[base:base + 64, :], y_pss[h][0:64, :],
                                             bc_sb[0:64, :])
                        # stage each head's half as soon as it is normalized
                        nc.sync.dma_start(ag_in[base:base + 64, :], yt[base:base + 64, :])
                    nc.gpsimd.collective_compute(
                        "AllGather", mybir.AluOpType.bypass,
                        ins=[ag_in[:]], outs=[ag_out[:]], replica_groups=RG)
                    for half in range(2):
                        g = p_yg.tile([128, 512], dt.bfloat16, tag="yg",
                                      name=f"yg{n}_{2 * hp + half}")
                        nc.sync.dma_start(g[:], ag_out[half * 128:(half + 1) * 128, :])
                        yg_tiles.setdefault(n, []).append(g)

                def emit_c_chunk(n, co):
                    o_ps = pp_wk.tile([128, 1024], dt.float32, tag="wk")
                    for p in range(8):
                        nc.tensor.matmul(o_ps[:, 0:512], wp_sb[p][:, co * 128:(co + 1) * 128],
                                         yg_tiles[n][p][:], start=(p == 0), stop=(p == 7))
                    o_sb = p_out.tile([128, 512], dt.float32)
                    nc.vector.tensor_scalar_add(o_sb[:], o_ps[:, 0:512], bp_sb[:, co:co + 1])
                    nc.sync.dma_start(out_e[co * 128:(co + 1) * 128, n * 512:(n + 1) * 512],
                                      o_sb[:])

                def emit_filler(f):
                    if f[0] == "q":
                        emit_q_slice(f[1], f[2])
                    else:
                        emit_c_chunk(f[1], f[2])

                for step in range(5):
                    bn = step - 1
                    qn = step if step < NQ else -1
                    cn = step - 2
                    fillers = []
                    if qn >= 0:
                        fillers += [("q", qn, mq) for mq in range(4)]
                    if cn >= 0:
                        # on the last step hold back two projection chunks to
                        # fill the PE while the final AllGather is in flight
                        ncos = 2 if step == 4 else 4
                        fillers += [("c", cn, co) for co in range(ncos)]
                    if bn < 0:
                        for f in fillers:
                            emit_filler(f)
                        continue
                    pairs_total = (2 * bn + 2) * 4
                    k = 0
                    fi = 0
                    for hp in range(4):
                        npair = 2 * bn + 2
                        for j in range(npair):
                            emit_scores_pair(bn, hp, j)
                            while fi < len(fillers) and fi * pairs_total < (k + 1) * len(fillers):
                                emit_filler(fillers[fi])
                                fi += 1
                            if j >= 1:
                                emit_avs_pair(bn, hp, j - 1)
                            k += 1
                        emit_avs_pair(bn, hp, npair - 1)
                        emit_norm(bn, hp)
                    while fi < len(fillers):
                        emit_filler(fillers[fi])
                        fi += 1

                # epilogue: the held-back chunk-2 projections (independent of
                # the final AllGather) fill the PE while it is in flight, then
                # chunk 3's projection. Pieces 0..5 emit piece-major; the last
                # two pieces run co-major so each co's bias-add + store
                # pipelines as soon as its group stops.
                emit_c_chunk(2, 2)
                emit_c_chunk(2, 3)
                n = 3
                opsA = pp_wk.tile([128, 1024], dt.float32, tag="wk")
                opsB = pp_wk.tile([128, 1024], dt.float32, tag="wk")
                regions = [opsA[:, 0:512], opsA[:, 512:1024],
                           opsB[:, 0:512], opsB[:, 512:1024]]
                for p in range(6):
                    for co in range(4):
                        nc.tensor.matmul(regions[co], wp_sb[p][:, co * 128:(co + 1) * 128],
                                         yg_tiles[n][p][:], start=(p == 0), stop=False)
                for co in range(4):
                    for p in (6, 7):
                        nc.tensor.matmul(regions[co], wp_sb[p][:, co * 128:(co + 1) * 128],
                                         yg_tiles[n][p][:], start=False, stop=(p == 7))
                    o_sb = p_out.tile([128, 512], dt.float32)
                    nc.vector.tensor_scalar_add(o_sb[:], regions[co], bp_sb[:, co:co + 1])
                    nc.sync.dma_start(out_e[co * 128:(co + 1) * 128, n * 512:(n + 1) * 512],
                                      o_sb[:])

    nc.finalize()
    return nc


def _get_nc():
    if "nc" not in _CACHE:
        _CACHE["nc"] = _build_nc()
    return _CACHE["nc"]


def _make_in_maps(x, W_attn, b_attn, W_proj, b_proj):
    x = np.asarray(x, dtype=np.float32)
    W_attn = np.asarray(W_attn, dtype=np.float32)
    b_attn = np.asarray(b_attn, dtype=np.float32)
    W_proj = np.asarray(W_proj, dtype=np.float32)
    b_proj = np.asarray(b_proj, dtype=np.float32)

    in_maps = []
    for core in range(8):
        b, hg = core // 2, core % 2
        lo, hi = hg * CL, (hg + 1) * CL
        wq = W_attn[:, lo:hi]
        wk = W_attn[:, C + lo:C + hi]
        wv = W_attn[:, 2 * C + lo:2 * C + hi]
        bq = b_attn[lo:hi]
        bk = b_attn[C + lo:C + hi]
        bv = b_attn[2 * C + lo:2 * C + hi]
        # permuted W_proj rows: per hp, the even core's two heads then the
        # odd core's two heads (matches AllGather piece arrival order);
        # columns are this core's half of the output channels.
        wp_rows = [W_proj[128 * hp + 512 * par:128 * hp + 512 * par + 128, lo:hi]
                   for hp in range(4) for par in range(2)]
        in_maps.append({
            "xT": np.ascontiguousarray(x[b].T.astype(BF16)),
            "wkv": np.ascontiguousarray(
                np.concatenate([wk, wv], axis=1).astype(BF16)),
            "wq": np.ascontiguousarray(wq.astype(BF16)),
            "wperm": np.ascontiguousarray(
                np.concatenate(wp_rows, axis=0).astype(BF16)),
            "bqk": np.ascontiguousarray(np.concatenate([bq, bk]).reshape(8, 128).T),
            "bvr": np.ascontiguousarray(bv.reshape(1, CL).astype(BF16)),
            "bproj": np.ascontiguousarray(b_proj[lo:hi].reshape(4, 128).T),
        })
    return in_maps


def _assemble(results):
    out = np.empty((B, T, C), dtype=np.float32)
    outT = np.empty((C, T), dtype=np.float32)
    for b in range(B):
        outT[0:512, :] = results[2 * b]["out"]
        outT[512:1024, :] = results[2 * b + 1]["out"]
        out[b] = outT.T
    return out


def run(trace=False, **inputs):
    nc = _get_nc()
    in_maps = _make_in_maps(**inputs)
    kw = {}
    if trace:
        kw = dict(trace=True, trace_cores=[0])
    res = run_bass_kernel_spmd(nc, in_maps, list(range(8)), **kw)
    return _assemble(res.results), res


def kernel(**inputs) -> np.ndarray:
    out, _ = run(trace=False, **inputs)
    return out


# revision 24
# speedup vs baseline: 1.0779x; 1.0395x over previous
"""Causal self-attention (B=4, T=2048, C=1024, H=16) on 8 TRN2 NeuronCores.

Sharding: core = 2*b + hg  (b = batch 0..3, hg = head-group 0..1, 8 heads each).
Datapath is bf16 matmuls with fp32 PSUM/softmax/normalization:
  prologue: k^T and v (with an interleaved ones-column per head for softmax
            denominators) for all 16 key chunks; x stays resident in SBUF.
  main loop over query chunks n: scores^T = k^T.T @ q^T per head pair
            (row-groups 0/64 pack the two heads), exp on ACT straight from
            PSUM, causal triangle via gpsimd affine_select, AV matmul with
            appended ones column, deferred per-query normalization; the q^T
            projection for chunk n+1 and the output projection for chunk n-1
            are interleaved into the same loop to keep the PE array dense.
  output:   after each head-pair norm the y piece [128,512] is AllGathered
            across the batch pair; each core then computes its own half of
            the output CHANNELS (full 1024-row contraction, permuted W_proj
            prepared host-side) and writes straight to out^T in DRAM. No
            ReduceScatter; only the final 128KB AllGather is on the
            critical path.
Host side transposes x per batch on the way in and reassembles/transposes the
output on the way out.
"""
import numpy as np
import ml_dtypes
from contextlib import ExitStack

import concourse.bass as bass
from concourse import bacc, mybir
from concourse.tile import TileContext
from concourse.bass_utils import run_bass_kernel_spmd

dt = mybir.dt
AF = mybir.ActivationFunctionType
BF16 = ml_dtypes.bfloat16

B, T, C, H = 4, 2048, 1024, 16
D = 64              # head dim
HL = 8              # heads per core
CL = HL * D         # 512 local channels
NQ = T // 512       # 4 query chunks of 512
NT = T // 128       # 16 key/time chunks of 128
SCALE = 1.0 / np.sqrt(D)

_CACHE = {}


def _build_nc():
    nc = bacc.Bacc("TRN2", target_bir_lowering=False, debug=False)

    xT_e = nc.declare_dram_parameter("xT", [C, T], dt.bfloat16, isOutput=False)
    wkv_e = nc.declare_dram_parameter("wkv", [C, 2 * CL], dt.bfloat16, isOutput=False)
    wq_e = nc.declare_dram_parameter("wq", [C, CL], dt.bfloat16, isOutput=False)
    wp_e = nc.declare_dram_parameter("wperm", [C, CL], dt.bfloat16, isOutput=False)
    bqk_e = nc.declare_dram_parameter("bqk", [128, 8], dt.float32, isOutput=False)
    bvr_e = nc.declare_dram_parameter("bvr", [1, CL], dt.bfloat16, isOutput=False)
    bp_e = nc.declare_dram_parameter("bproj", [128, 4], dt.float32, isOutput=False)
    out_e = nc.declare_dram_parameter("out", [CL, T], dt.float32, isOutput=True)

    RG = [[0, 1], [2, 3], [4, 5], [6, 7]]

    with TileContext(nc) as tc, nc.allow_low_precision("bf16 datapath by design"):
        with ExitStack() as top:
            p_cst = top.enter_context(tc.tile_pool(name="cst", bufs=1))
            p_x = top.enter_context(tc.tile_pool(name="xres", bufs=8))
            p_kt = top.enter_context(tc.tile_pool(name="kt", bufs=4))
            p_v = top.enter_context(tc.tile_pool(name="v", bufs=16))
            p_wq = top.enter_context(tc.tile_pool(name="wq", bufs=8))
            p_wp = top.enter_context(tc.tile_pool(name="wp", bufs=8))
            pp_wk = top.enter_context(tc.tile_pool(name="ppwk", bufs=2, space="PSUM"))
            pp_q = top.enter_context(tc.tile_pool(name="ppq", bufs=1, space="PSUM"))

            ones_f = p_cst.tile([128, 128], dt.float32)
            nc.gpsimd.memset(ones_f[:], 1.0)
            ones_bf = p_cst.tile([1, 128], dt.bfloat16)
            nc.vector.tensor_copy(ones_bf[:], ones_f[0:1, :])
            bqk_sb = p_cst.tile([128, 8], dt.float32)
            bp_sb = p_cst.tile([128, 4], dt.float32)
            bvr_sb = p_cst.tile([1, CL], dt.bfloat16)

            x_sb = [p_x.tile([128, T], dt.bfloat16, tag="x", name=f"x{c}")
                    for c in range(8)]
            kt_sb = [p_kt.tile([128, T], dt.bfloat16, tag="kt", name=f"ktt{i}")
                     for i in range(4)]
            v_sb = [p_v.tile([128, 8 * 65], dt.bfloat16, tag="v", name=f"vt{i}")
                    for i in range(NT)]

            # ---------------- prologue: k^T and v for all chunks ----------------
            with ExitStack() as pctx:
                p_wkv = pctx.enter_context(tc.tile_pool(name="wkv", bufs=8))
                # first matmul group's operands first (k-half weights + x cols
                # 0:512, pairwise) so PE starts ~immediately; v-half weights
                # and the rest of x stream behind.
                # descriptor-gen is ~0.6us per DMA on a queue: issue x on the
                # scalar queue and weights on sync concurrently, big DMAs only
                wkv_sb = []
                for c in range(8):
                    wt = p_wkv.tile([128, 1024], dt.bfloat16, tag="wkv", name=f"wkvt{c}")
                    nc.sync.dma_start(wt[:], wkv_e[c * 128:(c + 1) * 128, :])
                    wkv_sb.append(wt)
                    nc.scalar.dma_start(x_sb[c][:, 0:512], xT_e[c * 128:(c + 1) * 128, 0:512])
                nc.scalar.dma_start(bqk_sb[:], bqk_e[:])
                nc.scalar.dma_start(bp_sb[:], bp_e[:])
                nc.scalar.dma_start(bvr_sb[:], bvr_e[:])
                for c in range(8):
                    nc.sync.dma_start(x_sb[c][:, 512:2048],
                                      xT_e[c * 128:(c + 1) * 128, 512:2048])

                for n in range(NQ):
                    xof = n * 512
                    for mk in range(4):
                        ps_t = pp_wk.tile([128, 1024], dt.float32, tag="wk")
                        for c in range(8):
                            nc.tensor.matmul(ps_t[:, 0:512], wkv_sb[c][:, mk * 128:(mk + 1) * 128],
                                             x_sb[c][:, xof:xof + 512],
                                             start=(c == 0), stop=(c == 7))
                        nc.scalar.activation(kt_sb[mk][:, xof:xof + 512], ps_t[:, 0:512],
                                             AF.Identity, bias=bqk_sb[:, 4 + mk:5 + mk])
                    for tv in range(4):
                        ps_v = pp_q.tile([128, 512], dt.float32, tag="qv")
                        for c in range(8):
                            nc.tensor.matmul(ps_v[:], x_sb[c][:, xof + tv * 128:xof + (tv + 1) * 128],
                                             wkv_sb[c][:, 512:1024], start=(c == 0), stop=False)
                        nc.tensor.matmul(ps_v[:], ones_bf[:], bvr_sb[:], start=False, stop=True)
                        vt = v_sb[n * 4 + tv]
                        nc.scalar.activation(
                            vt[:].rearrange("p (h s) -> p h s", s=65)[:, :, 0:64],
                            ps_v[:].rearrange("p (h s) -> p h s", s=64),
                            AF.Copy)
                        nc.vector.tensor_copy(vt[:, 64:520:65], ones_f[:, 0:8])

            # ---------------- main loop ----------------
            wq_sb = []
            for c in range(8):
                wqt = p_wq.tile([128, CL], dt.bfloat16, tag="wq", name=f"wqt{c}")
                nc.sync.dma_start(wqt[:], wq_e[c * 128:(c + 1) * 128, :])
                wq_sb.append(wqt)
            wp_sb = []
            for p in range(8):
                wpt = p_wp.tile([128, CL], dt.bfloat16, tag="wp", name=f"wpt{p}")
                nc.sync.dma_start(wpt[:], wp_e[p * 128:(p + 1) * 128, :])
                wp_sb.append(wpt)

            with ExitStack() as bctx:
                p_q = bctx.enter_context(tc.tile_pool(name="q", bufs=8))
                p_att = bctx.enter_context(tc.tile_pool(name="att", bufs=8))
                p_y = bctx.enter_context(tc.tile_pool(name="yt", bufs=4))
                p_yg = bctx.enter_context(tc.tile_pool(name="yg", bufs=16))
                p_rec = bctx.enter_context(tc.tile_pool(name="rec", bufs=4))
                p_bc = bctx.enter_context(tc.tile_pool(name="bc", bufs=4))
                p_out = bctx.enter_context(tc.tile_pool(name="osb", bufs=4))
                pp_y = bctx.enter_context(tc.tile_pool(name="ppy", bufs=3, space="PSUM"))
                p_dram = bctx.enter_context(tc.tile_pool(name="agd", bufs=4, space="DRAM"))

                q_tiles = {}      # n -> [4 tiles of [128, 512]]
                yg_tiles = {}     # n -> [8 gathered y pieces [128, 512]]
                pair_store = {}   # (n, hp, j) -> (m0, m1, {h: (a_t, q0, q1)})
                ypss_store = {}   # (n, hp) -> {h: y_ps}

                def emit_q_slice(n, mq):
                    if mq == 0:
                        q_tiles[n] = []
                    ps_t = pp_q.tile([128, 512], dt.float32, tag="qv")
                    for c in range(8):
                        nc.tensor.matmul(ps_t[:], wq_sb[c][:, mq * 128:(mq + 1) * 128],
                                         x_sb[c][:, n * 512:(n + 1) * 512],
                                         start=(c == 0), stop=(c == 7))
                    qt = p_q.tile([128, 512], dt.bfloat16, tag="q", name=f"q{n}_{mq}")
                    nc.vector.tensor_scalar_add(qt[:], ps_t[:], bqk_sb[:, mq:mq + 1])
                    q_tiles[n].append(qt)

                def emit_scores_pair(n, hp, j):
                    h0, h1 = 2 * hp, 2 * hp + 1
                    if j == 0:
                        ypss_store[(n, hp)] = {
                            h: pp_y.tile([128, 512], dt.float32, tag="ypsum",
                                         name=f"yps{n}_{h}")
                            for h in (h0, h1)}
                    m0, m1 = 2 * j, 2 * j + 1
                    r0, r1 = m0 - 4 * n, m1 - 4 * n
                    q0 = 128 * r0 if r0 >= 0 else 0
                    q1 = 128 * r1 if r1 >= 0 else 0
                    entry = {}
                    for h in (h0, h1):
                        base = (h % 2) * 64
                        qt = q_tiles[n][h // 2]
                        kt = kt_sb[h // 2]
                        s_ps = pp_wk.tile([128, 1024], dt.float32, tag="wk")
                        nc.tensor.matmul(
                            s_ps[:, q0:512],
                            kt[base:base + 64, m0 * 128:(m0 + 1) * 128],
                            qt[base:base + 64, q0:512],
                            start=True, stop=True)
                        nc.tensor.matmul(
                            s_ps[:, 512 + q1:1024],
                            kt[base:base + 64, m1 * 128:(m1 + 1) * 128],
                            qt[base:base + 64, q1:512],
                            start=True, stop=True)
                        a_t = p_att.tile([128, 1024], dt.bfloat16, tag="att",
                                         name=f"a{n}_{hp}_{j}_{h}")
                        nc.scalar.activation(a_t[:, q0:1024], s_ps[:, q0:1024],
                                             AF.Exp, scale=float(SCALE))
                        if r0 >= 0:
                            nc.gpsimd.affine_select(
                                out=a_t[:, q0:q0 + 128], in_=a_t[:, q0:q0 + 128],
                                compare_op=mybir.AluOpType.is_ge, fill=0.0, base=0,
                                pattern=[[1, 128]], channel_multiplier=-1)
                        if r1 >= 0:
                            nc.gpsimd.affine_select(
                                out=a_t[:, 512 + q1:512 + q1 + 128],
                                in_=a_t[:, 512 + q1:512 + q1 + 128],
                                compare_op=mybir.AluOpType.is_ge, fill=0.0, base=0,
                                pattern=[[1, 128]], channel_multiplier=-1)
                        entry[h] = (a_t, q0, q1)
                    pair_store[(n, hp, j)] = (m0, m1, entry)

                def emit_avs_pair(n, hp, j):
                    m_max = 4 * n + 4
                    h0, h1 = 2 * hp, 2 * hp + 1
                    y_pss = ypss_store[(n, hp)]
                    m0, m1, entry = pair_store.pop((n, hp, j))
                    for h in (h0, h1):
                        a_t, q0, q1 = entry[h]
                        nc.tensor.matmul(
                            y_pss[h][0:65, q0:512],
                            v_sb[m0][:, h * 65:h * 65 + 65],
                            a_t[:, q0:512],
                            start=(m0 == 0), stop=False)
                        nc.tensor.matmul(
                            y_pss[h][0:65, q1:512],
                            v_sb[m1][:, h * 65:h * 65 + 65],
                            a_t[:, 512 + q1:1024],
                            start=False, stop=(m1 == m_max - 1))

                def emit_norm(n, hp):
                    h0, h1 = 2 * hp, 2 * hp + 1
                    y_pss = ypss_store.pop((n, hp))
                    yt = p_y.tile([128, 512], dt.bfloat16, tag="yt", name=f"yt{n}_{hp}")
                    ag_in = p_dram.tile([128, 512], dt.bfloat16, tag="agi",
                                        name=f"agi{n}_{hp}")
                    ag_out = p_dram.tile([256, 512], dt.bfloat16, tag="ago",
                                         name=f"ago{n}_{hp}")
                    # interleave so the gpsimd broadcasts overlap DVE work
                    recs, bcs = {}, {}
                    for h in (h0, h1):
                        rec_s = p_rec.tile([128, 512], dt.float32, tag="recs")
                        rec = p_rec.tile([128, 512], dt.float32, tag="rec")
                        nc.vector.tensor_copy(rec_s[0:1, :], y_pss[h][64:65, :])
                        nc.vector.reciprocal_approx_fast(out=rec[0:1, :], in_=rec_s[0:1, :])
                        recs[h] = rec
                        bcs[h] = p_bc.tile([128, 512], dt.float32, tag="bc",
                                           name=f"bc{n}_{h}")
                        nc.gpsimd.partition_broadcast(bcs[h][0:64, :], recs[h][0:1, :],
                                                      channels=64)
                    for h in (h0, h1):
                        base = (h % 2) * 64
                        nc.vector.tensor_mul(yt[base:base + 64, :], y_pss[h][0:64, :],
                                             bcs[h][0:64, :])
                        # stage each head's half as soon as it is normalized
                        nc.sync.dma_start(ag_in[base:base + 64, :], yt[base:base + 64, :])
                    nc.gpsimd.collective_compute(
                        "AllGather", mybir.AluOpType.bypass,
                        ins=[ag_in[:]], outs=[ag_out[:]], replica_groups=RG)
                    for half in range(2):
                        g = p_yg.tile([128, 512], dt.bfloat16, tag="yg",
                                      name=f"yg{n}_{2 * hp + half}")
                        nc.sync.dma_start(g[:], ag_out[half * 128:(half + 1) * 128, :])
                        yg_tiles.setdefault(n, []).append(g)

                def emit_c_chunk(n, co):
                    o_ps = pp_wk.tile([128, 1024], dt.float32, tag="wk")
                    for p in range(8):
                        nc.tensor.matmul(o_ps[:, 0:512], wp_sb[p][:, co * 128:(co + 1) * 128],
                                         yg_tiles[n][p][:], start=(p == 0), stop=(p == 7))
                    o_sb = p_out.tile([128, 512], dt.float32)
                    nc.vector.tensor_scalar_add(o_sb[:], o_ps[:, 0:512], bp_sb[:, co:co + 1])
                    nc.sync.dma_start(out_e[co * 128:(co + 1) * 128, n * 512:(n + 1) * 512],
                                      o_sb[:])

                def emit_filler(f):
                    if f[0] == "q":
                        emit_q_slice(f[1], f[2])
                    else:
                        emit_c_chunk(f[1], f[2])

                for step in range(5):
                    bn = step - 1
                    qn = step if step < NQ else -1
                    cn = step - 2
                    fillers = []
                    if qn >= 0:
                        fillers += [("q", qn, mq) for mq in range(4)]
                    if cn >= 0 and step < 4:
                        fillers += [("c", cn, co) for co in range(4)]
                    if bn < 0:
                        for f in fillers:
                            emit_filler(f)
                        continue
                    pairs_total = (2 * bn + 2) * 4
                    k = 0
                    fi = 0
                    for hp in range(4):
                        npair = 2 * bn + 2
                        for j in range(npair):
                            emit_scores_pair(bn, hp, j)
                            while fi < len(fillers) and fi * pairs_total < (k + 1) * len(fillers):
                                emit_filler(fillers[fi])
                                fi += 1
                            if j >= 1:
                                emit_avs_pair(bn, hp, j - 1)
                            k += 1
                        emit_avs_pair(bn, hp, npair - 1)
                        emit_norm(bn, hp)
                    while fi < len(fillers):
                        emit_filler(fillers[fi])
                        fi += 1

                # epilogue: the held-back chunk-2 projections (independent of
                # the final AllGather) fill the PE while it is in flight, then
                # chunk 3's projection. Pieces 0..5 emit piece-major; the last
                # two pieces run co-major so each co's bias-add + store
                # pipelines as soon as its group stops.
                for co in range(4):
                    emit_c_chunk(2, co)
                n = 3
                opsA = pp_wk.tile([128, 1024], dt.float32, tag="wk")
                opsB = pp_wk.tile([128, 1024], dt.float32, tag="wk")
                regions = [opsA[:, 0:512], opsA[:, 512:1024],
                           opsB[:, 0:512], opsB[:, 512:1024]]
                for p in range(6):
                    for co in range(4):
                        nc.tensor.matmul(regions[co], wp_sb[p][:, co * 128:(co + 1) * 128],
                                         yg_tiles[n][p][:], start=(p == 0), stop=False)
                for co in range(4):
                    for p in (6, 7):
                        nc.tensor.matmul(regions[co], wp_sb[p][:, co * 128:(co + 1) * 128],
                                         yg_tiles[n][p][:], start=False, stop=(p == 7))
                    o_sb = p_out.tile([128, 512], dt.float32)
                    nc.vector.tensor_scalar_add(o_sb[:], regions[co], bp_sb[:, co:co + 1])
                    nc.sync.dma_start(out_e[co * 128:(co + 1) * 128, n * 512:(n + 1) * 512],
                                      o_sb[:])

    nc.finalize()
    return nc


def _get_nc():
    if "nc" not in _CACHE:
        _CACHE["nc"] = _build_nc()
    return _CACHE["nc"]


def _make_in_maps(x, W_attn, b_attn, W_proj, b_proj):
    x = np.asarray(x, dtype=np.float32)
    W_attn = np.asarray(W_attn, dtype=np.float32)
    b_attn = np.asarray(b_attn, dtype=np.float32)
    W_proj = np.asarray(W_proj, dtype=np.float32)
    b_proj = np.asarray(b_proj, dtype=np.float32)

    in_maps = []
    for core in range(8):
        b, hg = core // 2, core % 2
        lo, hi = hg * CL, (hg + 1) * CL
        wq = W_attn[:, lo:hi]
        wk = W_attn[:, C + lo:C + hi]
        wv = W_attn[:, 2 * C + lo:2 * C + hi]
        bq = b_attn[lo:hi]
        bk = b_attn[C + lo:C + hi]
        bv = b_attn[2 * C + lo:2 * C + hi]
        # permuted W_proj rows: per hp, the even core's two heads then the
        # odd core's two heads (matches AllGather piece arrival order);
        # columns are this core's half of the output channels.
        wp_rows = [W_proj[128 * hp + 512 * par:128 * hp + 512 * par + 128, lo:hi]
                   for hp in range(4) for par in range(2)]
        in_maps.append({
            "xT": np.ascontiguousarray(x[b].T.astype(BF16)),
            "wkv": np.ascontiguousarray(
                np.concatenate([wk, wv], axis=1).astype(BF16)),
            "wq": np.ascontiguousarray(wq.astype(BF16)),
            "wperm": np.ascontiguousarray(
                np.concatenate(wp_rows, axis=0).astype(BF16)),
            "bqk": np.ascontiguousarray(np.concatenate([bq, bk]).reshape(8, 128).T),
            "bvr": np.ascontiguousarray(bv.reshape(1, CL).astype(BF16)),
            "bproj": np.ascontiguousarray(b_proj[lo:hi].reshape(4, 128).T),
        })
    return in_maps


def _assemble(results):
    out = np.empty((B, T, C), dtype=np.float32)
    outT = np.empty((C, T), dtype=np.float32)
    for b in range(B):
        outT[0:512, :] = results[2 * b]["out"]
        outT[512:1024, :] = results[2 * b + 1]["out"]
        out[b] = outT.T
    return out


def run(trace=False, **inputs):
    nc = _get_nc()
    in_maps = _make_in_maps(**inputs)
    kw = {}
    if trace:
        kw = dict(trace=True, trace_cores=[0])
    res = run_bass_kernel_spmd(nc, in_maps, list(range(8)), **kw)
    return _assemble(res.results), res


def kernel(**inputs) -> np.ndarray:
    out, _ = run(trace=False, **inputs)
    return out


# revision 31
# speedup vs baseline: 1.0941x; 1.0150x over previous
"""Causal self-attention (B=4, T=2048, C=1024, H=16) on 8 TRN2 NeuronCores.

Sharding: core = 2*b + hg  (b = batch 0..3, hg = head-group 0..1, 8 heads each).
Datapath is bf16 matmuls with fp32 PSUM/softmax/normalization:
  prologue: k^T and v (with an interleaved ones-column per head for softmax
            denominators) for all 16 key chunks; x stays resident in SBUF.
  main loop over query chunks n: scores^T = k^T.T @ q^T per head pair
            (row-groups 0/64 pack the two heads), exp on ACT straight from
            PSUM, causal triangle via gpsimd affine_select, AV matmul with
            appended ones column, deferred per-query normalization; the q^T
            projection for chunk n+1 and the output projection for chunk n-1
            are interleaved into the same loop to keep the PE array dense.
  output:   after each head-pair norm the y piece [128,512] is AllGathered
            across the batch pair; each core then computes its own half of
            the output CHANNELS (full 1024-row contraction, permuted W_proj
            prepared host-side) and writes straight to out^T in DRAM. No
            ReduceScatter; only the final 128KB AllGather is on the
            critical path.
Host side transposes x per batch on the way in and reassembles/transposes the
output on the way out.
"""
import numpy as np
import ml_dtypes
from contextlib import ExitStack

import concourse.bass as bass
from concourse import bacc, mybir
from concourse.tile import TileContext
from concourse.bass_utils import run_bass_kernel_spmd

dt = mybir.dt
AF = mybir.ActivationFunctionType
BF16 = ml_dtypes.bfloat16

B, T, C, H = 4, 2048, 1024, 16
D = 64              # head dim
HL = 8              # heads per core
CL = HL * D         # 512 local channels
NQ = T // 512       # 4 query chunks of 512
NT = T // 128       # 16 key/time chunks of 128
SCALE = 1.0 / np.sqrt(D)

_CACHE = {}


def _build_nc():
    nc = bacc.Bacc("TRN2", target_bir_lowering=False, debug=False)

    xT_e = nc.declare_dram_parameter("xT", [C, T], dt.bfloat16, isOutput=False)
    wkv_e = nc.declare_dram_parameter("wkv", [C, 2 * CL], dt.bfloat16, isOutput=False)
    wq_e = nc.declare_dram_parameter("wq", [C, CL], dt.bfloat16, isOutput=False)
    wp_e = nc.declare_dram_parameter("wperm", [C, CL], dt.bfloat16, isOutput=False)
    bqk_e = nc.declare_dram_parameter("bqk", [128, 8], dt.float32, isOutput=False)
    bvr_e = nc.declare_dram_parameter("bvr", [1, CL], dt.bfloat16, isOutput=False)
    bp_e = nc.declare_dram_parameter("bproj", [128, 4], dt.float32, isOutput=False)
    out_e = nc.declare_dram_parameter("out", [CL, T], dt.float32, isOutput=True)

    RG = [[0, 1], [2, 3], [4, 5], [6, 7]]

    with TileContext(nc) as tc, nc.allow_low_precision("bf16 datapath by design"):
        with ExitStack() as top:
            p_cst = top.enter_context(tc.tile_pool(name="cst", bufs=1))
            p_x = top.enter_context(tc.tile_pool(name="xres", bufs=8))
            p_kt = top.enter_context(tc.tile_pool(name="kt", bufs=4))
            p_v = top.enter_context(tc.tile_pool(name="v", bufs=16))
            p_wq = top.enter_context(tc.tile_pool(name="wq", bufs=8))
            p_wp = top.enter_context(tc.tile_pool(name="wp", bufs=8))
            pp_wk = top.enter_context(tc.tile_pool(name="ppwk", bufs=2, space="PSUM"))
            pp_q = top.enter_context(tc.tile_pool(name="ppq", bufs=1, space="PSUM"))

            ones_f = p_cst.tile([128, 128], dt.float32)
            nc.gpsimd.memset(ones_f[:], 1.0)
            ones_bf = p_cst.tile([1, 128], dt.bfloat16)
            nc.vector.tensor_copy(ones_bf[:], ones_f[0:1, :])
            bqk_sb = p_cst.tile([128, 8], dt.float32)
            bp_sb = p_cst.tile([128, 4], dt.float32)
            bvr_sb = p_cst.tile([1, CL], dt.bfloat16)

            x_sb = [p_x.tile([128, T], dt.bfloat16, tag="x", name=f"x{c}")
                    for c in range(8)]
            kt_sb = [p_kt.tile([128, T], dt.bfloat16, tag="kt", name=f"ktt{i}")
                     for i in range(4)]
            v_sb = [p_v.tile([128, 8 * 65], dt.bfloat16, tag="v", name=f"vt{i}")
                    for i in range(NT)]

            # ---------------- prologue: k^T and v for all chunks ----------------
            with ExitStack() as pctx:
                p_wkv = pctx.enter_context(tc.tile_pool(name="wkv", bufs=8))
                # first matmul group's operands first (k-half weights + x cols
                # 0:512, pairwise) so PE starts ~immediately; v-half weights
                # and the rest of x stream behind.
                # descriptor-gen is ~0.6us per DMA on a queue: issue x on the
                # scalar queue and weights on sync concurrently, big DMAs only
                wkv_sb = []
                for c in range(8):
                    wt = p_wkv.tile([128, 1024], dt.bfloat16, tag="wkv", name=f"wkvt{c}")
                    nc.sync.dma_start(wt[:], wkv_e[c * 128:(c + 1) * 128, :])
                    wkv_sb.append(wt)
                    nc.scalar.dma_start(x_sb[c][:, 0:512], xT_e[c * 128:(c + 1) * 128, 0:512])
                nc.scalar.dma_start(bqk_sb[:], bqk_e[:])
                nc.scalar.dma_start(bp_sb[:], bp_e[:])
                nc.scalar.dma_start(bvr_sb[:], bvr_e[:])
                for c in range(8):
                    nc.sync.dma_start(x_sb[c][:, 512:2048],
                                      xT_e[c * 128:(c + 1) * 128, 512:2048])

                for n in range(NQ):
                    xof = n * 512
                    for mk in range(4):
                        ps_t = pp_wk.tile([128, 1024], dt.float32, tag="wk")
                        for c in range(8):
                            nc.tensor.matmul(ps_t[:, 0:512], wkv_sb[c][:, mk * 128:(mk + 1) * 128],
                                             x_sb[c][:, xof:xof + 512],
                                             start=(c == 0), stop=(c == 7))
                        nc.scalar.activation(kt_sb[mk][:, xof:xof + 512], ps_t[:, 0:512],
                                             AF.Identity, bias=bqk_sb[:, 4 + mk:5 + mk])
                    for tv in range(4):
                        ps_v = pp_q.tile([128, 512], dt.float32, tag="qv")
                        for c in range(8):
                            nc.tensor.matmul(ps_v[:], x_sb[c][:, xof + tv * 128:xof + (tv + 1) * 128],
                                             wkv_sb[c][:, 512:1024], start=(c == 0), stop=False)
                        nc.tensor.matmul(ps_v[:], ones_bf[:], bvr_sb[:], start=False, stop=True)
                        vt = v_sb[n * 4 + tv]
                        nc.scalar.activation(
                            vt[:].rearrange("p (h s) -> p h s", s=65)[:, :, 0:64],
                            ps_v[:].rearrange("p (h s) -> p h s", s=64),
                            AF.Copy)
                        nc.vector.tensor_copy(vt[:, 64:520:65], ones_f[:, 0:8])

            # ---------------- main loop ----------------
            wq_sb = []
            for c in range(8):
                wqt = p_wq.tile([128, CL], dt.bfloat16, tag="wq", name=f"wqt{c}")
                nc.sync.dma_start(wqt[:], wq_e[c * 128:(c + 1) * 128, :])
                wq_sb.append(wqt)
            wp_sb = []
            for p in range(8):
                wpt = p_wp.tile([128, CL], dt.bfloat16, tag="wp", name=f"wpt{p}")
                nc.sync.dma_start(wpt[:], wp_e[p * 128:(p + 1) * 128, :])
                wp_sb.append(wpt)

            with ExitStack() as bctx:
                p_q = bctx.enter_context(tc.tile_pool(name="q", bufs=8))
                p_att = bctx.enter_context(tc.tile_pool(name="att", bufs=8))
                p_y = bctx.enter_context(tc.tile_pool(name="yt", bufs=4))
                p_yg = bctx.enter_context(tc.tile_pool(name="yg", bufs=16))
                p_rec = bctx.enter_context(tc.tile_pool(name="rec", bufs=4))
                p_bc = bctx.enter_context(tc.tile_pool(name="bc", bufs=4))
                p_out = bctx.enter_context(tc.tile_pool(name="osb", bufs=4))
                pp_y = bctx.enter_context(tc.tile_pool(name="ppy", bufs=3, space="PSUM"))
                p_dram = bctx.enter_context(tc.tile_pool(name="agd", bufs=4, space="DRAM"))

                q_tiles = {}      # n -> [4 tiles of [128, 512]]
                yg_tiles = {}     # n -> [8 gathered y pieces [128, 512]]
                pair_store = {}   # (n, hp, j) -> (m0, m1, {h: (a_t, q0, q1)})
                ypss_store = {}   # (n, hp) -> {h: y_ps}
                pending_ag = []   # deferred AllGather closures

                def flush_ags():
                    while pending_ag:
                        pending_ag.pop(0)()

                def emit_q_slice(n, mq):
                    if mq == 0:
                        q_tiles[n] = []
                    ps_t = pp_q.tile([128, 512], dt.float32, tag="qv")
                    for c in range(8):
                        nc.tensor.matmul(ps_t[:], wq_sb[c][:, mq * 128:(mq + 1) * 128],
                                         x_sb[c][:, n * 512:(n + 1) * 512],
                                         start=(c == 0), stop=(c == 7))
                    qt = p_q.tile([128, 512], dt.bfloat16, tag="q", name=f"q{n}_{mq}")
                    nc.vector.tensor_scalar_add(qt[:], ps_t[:], bqk_sb[:, mq:mq + 1])
                    q_tiles[n].append(qt)

                def emit_scores_pair(n, hp, j):
                    if j == 1:
                        # previous head-pairs' AG triggers go here, after this
                        # pair's predecessors' affine_selects are on the queue
                        flush_ags()
                    h0, h1 = 2 * hp, 2 * hp + 1
                    if j == 0:
                        ypss_store[(n, hp)] = {
                            h: pp_y.tile([128, 512], dt.float32, tag="ypsum",
                                         name=f"yps{n}_{h}")
                            for h in (h0, h1)}
                    m0, m1 = 2 * j, 2 * j + 1
                    r0, r1 = m0 - 4 * n, m1 - 4 * n
                    q0 = 128 * r0 if r0 >= 0 else 0
                    q1 = 128 * r1 if r1 >= 0 else 0
                    entry = {}
                    for h in (h0, h1):
                        base = (h % 2) * 64
                        qt = q_tiles[n][h // 2]
                        kt = kt_sb[h // 2]
                        s_ps = pp_wk.tile([128, 1024], dt.float32, tag="wk")
                        nc.tensor.matmul(
                            s_ps[:, q0:512],
                            kt[base:base + 64, m0 * 128:(m0 + 1) * 128],
                            qt[base:base + 64, q0:512],
                            start=True, stop=True)
                        nc.tensor.matmul(
                            s_ps[:, 512 + q1:1024],
                            kt[base:base + 64, m1 * 128:(m1 + 1) * 128],
                            qt[base:base + 64, q1:512],
                            start=True, stop=True)
                        a_t = p_att.tile([128, 1024], dt.bfloat16, tag="att",
                                         name=f"a{n}_{hp}_{j}_{h}")
                        nc.scalar.activation(a_t[:, q0:1024], s_ps[:, q0:1024],
                                             AF.Exp, scale=float(SCALE))
                        if r0 >= 0:
                            nc.gpsimd.affine_select(
                                out=a_t[:, q0:q0 + 128], in_=a_t[:, q0:q0 + 128],
                                compare_op=mybir.AluOpType.is_ge, fill=0.0, base=0,
                                pattern=[[1, 128]], channel_multiplier=-1)
                        if r1 >= 0:
                            nc.gpsimd.affine_select(
                                out=a_t[:, 512 + q1:512 + q1 + 128],
                                in_=a_t[:, 512 + q1:512 + q1 + 128],
                                compare_op=mybir.AluOpType.is_ge, fill=0.0, base=0,
                                pattern=[[1, 128]], channel_multiplier=-1)
                        entry[h] = (a_t, q0, q1)
                    pair_store[(n, hp, j)] = (m0, m1, entry)

                def emit_avs_pair(n, hp, j):
                    m_max = 4 * n + 4
                    h0, h1 = 2 * hp, 2 * hp + 1
                    y_pss = ypss_store[(n, hp)]
                    m0, m1, entry = pair_store.pop((n, hp, j))
                    for h in (h0, h1):
                        a_t, q0, q1 = entry[h]
                        nc.tensor.matmul(
                            y_pss[h][0:65, q0:512],
                            v_sb[m0][:, h * 65:h * 65 + 65],
                            a_t[:, q0:512],
                            start=(m0 == 0), stop=False)
                        nc.tensor.matmul(
                            y_pss[h][0:65, q1:512],
                            v_sb[m1][:, h * 65:h * 65 + 65],
                            a_t[:, 512 + q1:1024],
                            start=False, stop=(m1 == m_max - 1))

                def emit_norm(n, hp):
                    h0, h1 = 2 * hp, 2 * hp + 1
                    y_pss = ypss_store.pop((n, hp))
                    yt = p_y.tile([128, 512], dt.bfloat16, tag="yt", name=f"yt{n}_{hp}")
                    ag_in = p_dram.tile([128, 512], dt.bfloat16, tag="agi",
                                        name=f"agi{n}_{hp}")
                    ag_out = p_dram.tile([256, 512], dt.bfloat16, tag="ago",
                                         name=f"ago{n}_{hp}")
                    # interleave so the gpsimd broadcasts overlap DVE work
                    recs, bcs = {}, {}
                    for h in (h0, h1):
                        rec_s = p_rec.tile([128, 512], dt.float32, tag="recs")
                        rec = p_rec.tile([128, 512], dt.float32, tag="rec")
                        nc.vector.tensor_copy(rec_s[0:1, :], y_pss[h][64:65, :])
                        nc.vector.reciprocal_approx_fast(out=rec[0:1, :], in_=rec_s[0:1, :])
                        recs[h] = rec
                        bcs[h] = p_bc.tile([128, 512], dt.float32, tag="bc",
                                           name=f"bc{n}_{h}")
                        nc.gpsimd.partition_broadcast(bcs[h][0:64, :], recs[h][0:1, :],
                                                      channels=64)
                    last = (n == 3 and hp == 3)
                    for h in (h0, h1):
                        base = (h % 2) * 64
                        nc.vector.tensor_mul(yt[base:base + 64, :], y_pss[h][0:64, :],
                                             bcs[h][0:64, :])
                        # stage each head's half as soon as it is normalized;
                        # the last norm stages via ACT's queue to dodge
                        # head-of-line AG waits on sync
                        eng = nc.scalar if last else nc.sync
                        eng.dma_start(ag_in[base:base + 64, :], yt[base:base + 64, :])

                    def do_ag(n=n, hp=hp, ag_in=ag_in, ag_out=ag_out):
                        nc.gpsimd.collective_compute(
                            "AllGather", mybir.AluOpType.bypass,
                            ins=[ag_in[:]], outs=[ag_out[:]], replica_groups=RG)
                        for half in range(2):
                            g = p_yg.tile([128, 512], dt.bfloat16, tag="yg",
                                          name=f"yg{n}_{2 * hp + half}")
                            nc.sync.dma_start(g[:], ag_out[half * 128:(half + 1) * 128, :])
                            yg_tiles.setdefault(n, []).append(g)

                    # defer the AG trigger so its gpsimd wait doesn't block the
                    # next head-pair's affine_selects; the last one is critical
                    if last:
                        flush_ags()
                        do_ag()
                    else:
                        pending_ag.append(do_ag)

                def emit_c_chunk(n, co, on_act=False):
                    flush_ags()
                    o_ps = pp_wk.tile([128, 1024], dt.float32, tag="wk")
                    for p in range(8):
                        nc.tensor.matmul(o_ps[:, 0:512], wp_sb[p][:, co * 128:(co + 1) * 128],
                                         yg_tiles[n][p][:], start=(p == 0), stop=(p == 7))
                    o_sb = p_out.tile([128, 512], dt.float32)
                    # epilogue chunks add bias on ACT (idle there); mid-step
                    # fillers use DVE to keep the exp stream unblocked
                    if on_act:
                        nc.scalar.activation(o_sb[:], o_ps[:, 0:512], AF.Identity,
                                             bias=bp_sb[:, co:co + 1])
                    else:
                        nc.vector.tensor_scalar_add(o_sb[:], o_ps[:, 0:512], bp_sb[:, co:co + 1])
                    nc.sync.dma_start(out_e[co * 128:(co + 1) * 128, n * 512:(n + 1) * 512],
                                      o_sb[:])

                def emit_filler(f):
                    if f[0] == "q":
                        emit_q_slice(f[1], f[2])
                    else:
                        emit_c_chunk(f[1], f[2])

                for step in range(5):
                    bn = step - 1
                    qn = step if step < NQ else -1
                    cn = step - 2
                    fillers = []
                    if qn >= 0:
                        fillers += [("q", qn, mq) for mq in range(4)]
                    if cn >= 0 and step < 4:
                        fillers += [("c", cn, co) for co in range(4)]
                    if bn < 0:
                        for f in fillers:
                            emit_filler(f)
                        continue
                    pairs_total = (2 * bn + 2) * 4
                    k = 0
                    fi = 0
                    for hp in range(4):
                        npair = 2 * bn + 2
                        for j in range(npair):
                            emit_scores_pair(bn, hp, j)
                            while fi < len(fillers) and fi * pairs_total < (k + 1) * len(fillers):
                                emit_filler(fillers[fi])
                                fi += 1
                            if j >= 1:
                                emit_avs_pair(bn, hp, j - 1)
                            k += 1
                        emit_avs_pair(bn, hp, npair - 1)
                        emit_norm(bn, hp)
                    while fi < len(fillers):
                        emit_filler(fillers[fi])
                        fi += 1

                # epilogue: the held-back chunk-2 projections (independent of
                # the final AllGather) fill the PE while it is in flight, then
                # chunk 3's projection. Pieces 0..5 emit piece-major; the last
                # two pieces run co-major so each co's bias-add + store
                # pipelines as soon as its group stops.
                for co in range(4):
                    emit_c_chunk(2, co, on_act=True)
                n = 3
                opsA = pp_wk.tile([128, 1024], dt.float32, tag="wk")
                opsB = pp_wk.tile([128, 1024], dt.float32, tag="wk")
                regions = [opsA[:, 0:512], opsA[:, 512:1024],
                           opsB[:, 0:512], opsB[:, 512:1024]]
                for p in range(6):
                    for co in range(4):
                        nc.tensor.matmul(regions[co], wp_sb[p][:, co * 128:(co + 1) * 128],
                                         yg_tiles[n][p][:], start=(p == 0), stop=False)
                for co in range(4):
                    for p in (6, 7):
                        nc.tensor.matmul(regions[co], wp_sb[p][:, co * 128:(co + 1) * 128],
                                         yg_tiles[n][p][:], start=False, stop=(p == 7))
                    o_sb = p_out.tile([128, 512], dt.float32)
                    nc.scalar.activation(o_sb[:], regions[co], AF.Identity,
                                         bias=bp_sb[:, co:co + 1])
                    nc.sync.dma_start(out_e[co * 128:(co + 1) * 128, n * 512:(n + 1) * 512],
                                      o_sb[:])

    nc.finalize()
    return nc


def _get_nc():
    if "nc" not in _CACHE:
        _CACHE["nc"] = _build_nc()
    return _CACHE["nc"]


def _make_in_maps(x, W_attn, b_attn, W_proj, b_proj):
    x = np.asarray(x, dtype=np.float32)
    W_attn = np.asarray(W_attn, dtype=np.float32)
    b_attn = np.asarray(b_attn, dtype=np.float32)
    W_proj = np.asarray(W_proj, dtype=np.float32)
    b_proj = np.asarray(b_proj, dtype=np.float32)

    in_maps = []
    for core in range(8):
        b, hg = core // 2, core % 2
        lo, hi = hg * CL, (hg + 1) * CL
        wq = W_attn[:, lo:hi]
        wk = W_attn[:, C + lo:C + hi]
        wv = W_attn[:, 2 * C + lo:2 * C + hi]
        bq = b_attn[lo:hi]
        bk = b_attn[C + lo:C + hi]
        bv = b_attn[2 * C + lo:2 * C + hi]
        # permuted W_proj rows: per hp, the even core's two heads then the
        # odd core's two heads (matches AllGather piece arrival order);
        # columns are this core's half of the output channels.
        wp_rows = [W_proj[128 * hp + 512 * par:128 * hp + 512 * par + 128, lo:hi]
                   for hp in range(4) for par in range(2)]
        in_maps.append({
            "xT": np.ascontiguousarray(x[b].T.astype(BF16)),
            "wkv": np.ascontiguousarray(
                np.concatenate([wk, wv], axis=1).astype(BF16)),
            "wq": np.ascontiguousarray(wq.astype(BF16)),
            "wperm": np.ascontiguousarray(
                np.concatenate(wp_rows, axis=0).astype(BF16)),
            "bqk": np.ascontiguousarray(np.concatenate([bq, bk]).reshape(8, 128).T),
            "bvr": np.ascontiguousarray(bv.reshape(1, CL).astype(BF16)),
            "bproj": np.ascontiguousarray(b_proj[lo:hi].reshape(4, 128).T),
        })
    return in_maps


def _assemble(results):
    out = np.empty((B, T, C), dtype=np.float32)
    outT = np.empty((C, T), dtype=np.float32)
    for b in range(B):
        outT[0:512, :] = results[2 * b]["out"]
        outT[512:1024, :] = results[2 * b + 1]["out"]
        out[b] = outT.T
    return out


def run(trace=False, **inputs):
    nc = _get_nc()
    in_maps = _make_in_maps(**inputs)
    kw = {}
    if trace:
        kw = dict(trace=True, trace_cores=[0])
    res = run_bass_kernel_spmd(nc, in_maps, list(range(8)), **kw)
    return _assemble(res.results), res


def kernel(**inputs) -> np.ndarray:
    out, _ = run(trace=False, **inputs)
    return out


# revision 34
# speedup vs baseline: 1.1773x; 1.0761x over previous
"""Causal self-attention (B=4, T=2048, C=1024, H=16) on 8 TRN2 NeuronCores.

Sharding: core = 2*b + hg  (b = batch 0..3, hg = head-group 0..1, 8 heads each).
Datapath is bf16 matmuls with fp32 PSUM/softmax/normalization:
  prologue: k^T and v (with an interleaved ones-column per head for softmax
            denominators) for all 16 key chunks; x stays resident in SBUF.
  main loop over query chunks n: scores^T = k^T.T @ q^T per head pair
            (row-groups 0/64 pack the two heads), exp on ACT straight from
            PSUM, causal triangle via gpsimd affine_select, AV matmul with
            appended ones column, deferred per-query normalization; the q^T
            projection for chunk n+1 and the output projection for chunk n-1
            are interleaved into the same loop to keep the PE array dense.
  output:   after each head-pair norm the y piece [128,512] is AllGathered
            across the batch pair; each core then computes its own half of
            the output CHANNELS (full 1024-row contraction, permuted W_proj
            prepared host-side) and writes straight to out^T in DRAM. No
            ReduceScatter; only the final 128KB AllGather is on the
            critical path.
Host side transposes x per batch on the way in and reassembles/transposes the
output on the way out.
"""
import numpy as np
import ml_dtypes
from contextlib import ExitStack

import concourse.bass as bass
from concourse import bacc, mybir
from concourse.tile import TileContext
from concourse.bass_utils import run_bass_kernel_spmd

dt = mybir.dt
AF = mybir.ActivationFunctionType
BF16 = ml_dtypes.bfloat16

B, T, C, H = 4, 2048, 1024, 16
D = 64              # head dim
HL = 8              # heads per core
CL = HL * D         # 512 local channels
NQ = T // 512       # 4 query chunks of 512
NT = T // 128       # 16 key/time chunks of 128
SCALE = 1.0 / np.sqrt(D)

_CACHE = {}


def _build_nc():
    nc = bacc.Bacc("TRN2", target_bir_lowering=False, debug=False)

    xT_e = nc.declare_dram_parameter("xT", [C, T], dt.bfloat16, isOutput=False)
    wkv_e = nc.declare_dram_parameter("wkv", [C, 2 * CL], dt.bfloat16, isOutput=False)
    wq_e = nc.declare_dram_parameter("wq", [C, CL], dt.bfloat16, isOutput=False)
    wp_e = nc.declare_dram_parameter("wperm", [C, CL], dt.bfloat16, isOutput=False)
    bqk_e = nc.declare_dram_parameter("bqk", [128, 8], dt.float32, isOutput=False)
    bvr_e = nc.declare_dram_parameter("bvr", [1, CL], dt.bfloat16, isOutput=False)
    bp_e = nc.declare_dram_parameter("bproj", [128, 4], dt.float32, isOutput=False)
    out_e = nc.declare_dram_parameter("out", [CL, T], dt.float32, isOutput=True)

    RG = [[0, 1], [2, 3], [4, 5], [6, 7]]

    with TileContext(nc) as tc, nc.allow_low_precision("bf16 datapath by design"):
        with ExitStack() as top:
            p_cst = top.enter_context(tc.tile_pool(name="cst", bufs=1))
            p_x = top.enter_context(tc.tile_pool(name="xres", bufs=8))
            p_kt = top.enter_context(tc.tile_pool(name="kt", bufs=4))
            p_v = top.enter_context(tc.tile_pool(name="v", bufs=16))
            p_wq = top.enter_context(tc.tile_pool(name="wq", bufs=8))
            p_wp = top.enter_context(tc.tile_pool(name="wp", bufs=8))
            pp_wk = top.enter_context(tc.tile_pool(name="ppwk", bufs=2, space="PSUM"))
            pp_q = top.enter_context(tc.tile_pool(name="ppq", bufs=1, space="PSUM"))

            ones_f = p_cst.tile([128, 128], dt.float32)
            nc.gpsimd.memset(ones_f[:], 1.0)
            ones_bf = p_cst.tile([1, 128], dt.bfloat16)
            nc.vector.tensor_copy(ones_bf[:], ones_f[0:1, :])
            # lower-triangular causal mask: every diagonal block has equal
            # query/key offsets, so one constant serves all of them
            tri = p_cst.tile([128, 128], dt.bfloat16)
            nc.vector.tensor_copy(tri[:], ones_f[:])
            nc.gpsimd.affine_select(
                out=tri[:], in_=tri[:], compare_op=mybir.AluOpType.is_ge,
                fill=0.0, base=0, pattern=[[1, 128]], channel_multiplier=-1)
            bqk_sb = p_cst.tile([128, 8], dt.float32)
            bp_sb = p_cst.tile([128, 4], dt.float32)
            bvr_sb = p_cst.tile([1, CL], dt.bfloat16)

            x_sb = [p_x.tile([128, T], dt.bfloat16, tag="x", name=f"x{c}")
                    for c in range(8)]
            kt_sb = [p_kt.tile([128, T], dt.bfloat16, tag="kt", name=f"ktt{i}")
                     for i in range(4)]
            v_sb = [p_v.tile([128, 8 * 65], dt.bfloat16, tag="v", name=f"vt{i}")
                    for i in range(NT)]

            # ---------------- prologue: k^T and v for all chunks ----------------
            with ExitStack() as pctx:
                p_wkv = pctx.enter_context(tc.tile_pool(name="wkv", bufs=8))
                # first matmul group's operands first (k-half weights + x cols
                # 0:512, pairwise) so PE starts ~immediately; v-half weights
                # and the rest of x stream behind.
                # descriptor-gen is ~0.6us per DMA on a queue: issue x on the
                # scalar queue and weights on sync concurrently, big DMAs only
                wkv_sb = []
                for c in range(8):
                    wt = p_wkv.tile([128, 1024], dt.bfloat16, tag="wkv", name=f"wkvt{c}")
                    nc.sync.dma_start(wt[:, 0:512], wkv_e[c * 128:(c + 1) * 128, 0:512])
                    wkv_sb.append(wt)
                    nc.scalar.dma_start(x_sb[c][:, 0:512], xT_e[c * 128:(c + 1) * 128, 0:512])
                nc.scalar.dma_start(bqk_sb[:], bqk_e[:])
                nc.scalar.dma_start(bp_sb[:], bp_e[:])
                nc.scalar.dma_start(bvr_sb[:], bvr_e[:])
                for c in range(8):
                    nc.sync.dma_start(wkv_sb[c][:, 512:1024],
                                      wkv_e[c * 128:(c + 1) * 128, 512:1024])
                for c in range(8):
                    nc.sync.dma_start(x_sb[c][:, 512:2048],
                                      xT_e[c * 128:(c + 1) * 128, 512:2048])

                for n in range(NQ):
                    xof = n * 512
                    for mk in range(4):
                        ps_t = pp_wk.tile([128, 1024], dt.float32, tag="wk")
                        for c in range(8):
                            nc.tensor.matmul(ps_t[:, 0:512], wkv_sb[c][:, mk * 128:(mk + 1) * 128],
                                             x_sb[c][:, xof:xof + 512],
                                             start=(c == 0), stop=(c == 7))
                        nc.scalar.activation(kt_sb[mk][:, xof:xof + 512], ps_t[:, 0:512],
                                             AF.Identity, bias=bqk_sb[:, 4 + mk:5 + mk])
                    for tv in range(4):
                        ps_v = pp_q.tile([128, 512], dt.float32, tag="qv")
                        for c in range(8):
                            nc.tensor.matmul(ps_v[:], x_sb[c][:, xof + tv * 128:xof + (tv + 1) * 128],
                                             wkv_sb[c][:, 512:1024], start=(c == 0), stop=False)
                        nc.tensor.matmul(ps_v[:], ones_bf[:], bvr_sb[:], start=False, stop=True)
                        vt = v_sb[n * 4 + tv]
                        nc.scalar.activation(
                            vt[:].rearrange("p (h s) -> p h s", s=65)[:, :, 0:64],
                            ps_v[:].rearrange("p (h s) -> p h s", s=64),
                            AF.Copy)
                        nc.vector.tensor_copy(vt[:, 64:520:65], ones_f[:, 0:8])

            # ---------------- main loop ----------------
            wq_sb = []
            for c in range(8):
                wqt = p_wq.tile([128, CL], dt.bfloat16, tag="wq", name=f"wqt{c}")
                nc.sync.dma_start(wqt[:], wq_e[c * 128:(c + 1) * 128, :])
                wq_sb.append(wqt)
            wp_sb = []
            for p in range(8):
                wpt = p_wp.tile([128, CL], dt.bfloat16, tag="wp", name=f"wpt{p}")
                nc.sync.dma_start(wpt[:], wp_e[p * 128:(p + 1) * 128, :])
                wp_sb.append(wpt)

            with ExitStack() as bctx:
                p_q = bctx.enter_context(tc.tile_pool(name="q", bufs=8))
                p_att = bctx.enter_context(tc.tile_pool(name="att", bufs=8))
                p_y = bctx.enter_context(tc.tile_pool(name="yt", bufs=4))
                p_yg = bctx.enter_context(tc.tile_pool(name="yg", bufs=16))
                p_rec = bctx.enter_context(tc.tile_pool(name="rec", bufs=4))
                p_bc = bctx.enter_context(tc.tile_pool(name="bc", bufs=4))
                p_out = bctx.enter_context(tc.tile_pool(name="osb", bufs=4))
                pp_y = bctx.enter_context(tc.tile_pool(name="ppy", bufs=3, space="PSUM"))
                p_dram = bctx.enter_context(tc.tile_pool(name="agd", bufs=4, space="DRAM"))

                q_tiles = {}      # n -> [4 tiles of [128, 512]]
                yg_tiles = {}     # n -> [8 gathered y pieces [128, 512]]
                pair_store = {}   # (n, hp, j) -> (m0, m1, {h: (a_t, q0, q1)})
                ypss_store = {}   # (n, hp) -> {h: y_ps}
                pending_ag = []   # deferred AllGather closures

                def flush_ags():
                    while pending_ag:
                        pending_ag.pop(0)()

                def emit_q_slice(n, mq):
                    if mq == 0:
                        q_tiles[n] = []
                    ps_t = pp_q.tile([128, 512], dt.float32, tag="qv")
                    for c in range(8):
                        nc.tensor.matmul(ps_t[:], wq_sb[c][:, mq * 128:(mq + 1) * 128],
                                         x_sb[c][:, n * 512:(n + 1) * 512],
                                         start=(c == 0), stop=(c == 7))
                    qt = p_q.tile([128, 512], dt.bfloat16, tag="q", name=f"q{n}_{mq}")
                    nc.vector.tensor_scalar_add(qt[:], ps_t[:], bqk_sb[:, mq:mq + 1])
                    q_tiles[n].append(qt)

                def emit_scores_pair(n, hp, j):
                    if j == 1:
                        # previous head-pairs' AG triggers go here, after this
                        # pair's predecessors' affine_selects are on the queue
                        flush_ags()
                    h0, h1 = 2 * hp, 2 * hp + 1
                    if j == 0:
                        ypss_store[(n, hp)] = {
                            h: pp_y.tile([128, 512], dt.float32, tag="ypsum",
                                         name=f"yps{n}_{h}")
                            for h in (h0, h1)}
                    m0, m1 = 2 * j, 2 * j + 1
                    r0, r1 = m0 - 4 * n, m1 - 4 * n
                    q0 = 128 * r0 if r0 >= 0 else 0
                    q1 = 128 * r1 if r1 >= 0 else 0
                    entry = {}
                    for h in (h0, h1):
                        base = (h % 2) * 64
                        qt = q_tiles[n][h // 2]
                        kt = kt_sb[h // 2]
                        s_ps = pp_wk.tile([128, 1024], dt.float32, tag="wk")
                        nc.tensor.matmul(
                            s_ps[:, q0:512],
                            kt[base:base + 64, m0 * 128:(m0 + 1) * 128],
                            qt[base:base + 64, q0:512],
                            start=True, stop=True)
                        nc.tensor.matmul(
                            s_ps[:, 512 + q1:1024],
                            kt[base:base + 64, m1 * 128:(m1 + 1) * 128],
                            qt[base:base + 64, q1:512],
                            start=True, stop=True)
                        a_t = p_att.tile([128, 1024], dt.bfloat16, tag="att",
                                         name=f"a{n}_{hp}_{j}_{h}")
                        nc.scalar.activation(a_t[:, q0:1024], s_ps[:, q0:1024],
                                             AF.Exp, scale=float(SCALE))
                        if r0 >= 0:
                            nc.vector.tensor_mul(a_t[:, q0:q0 + 128],
                                                 a_t[:, q0:q0 + 128], tri[:])
                        if r1 >= 0:
                            nc.vector.tensor_mul(a_t[:, 512 + q1:512 + q1 + 128],
                                                 a_t[:, 512 + q1:512 + q1 + 128],
                                                 tri[:])
                        entry[h] = (a_t, q0, q1)
                    pair_store[(n, hp, j)] = (m0, m1, entry)

                def emit_avs_pair(n, hp, j):
                    m_max = 4 * n + 4
                    h0, h1 = 2 * hp, 2 * hp + 1
                    y_pss = ypss_store[(n, hp)]
                    m0, m1, entry = pair_store.pop((n, hp, j))
                    for h in (h0, h1):
                        a_t, q0, q1 = entry[h]
                        nc.tensor.matmul(
                            y_pss[h][0:65, q0:512],
                            v_sb[m0][:, h * 65:h * 65 + 65],
                            a_t[:, q0:512],
                            start=(m0 == 0), stop=False)
                        nc.tensor.matmul(
                            y_pss[h][0:65, q1:512],
                            v_sb[m1][:, h * 65:h * 65 + 65],
                            a_t[:, 512 + q1:1024],
                            start=False, stop=(m1 == m_max - 1))

                def emit_norm(n, hp):
                    h0, h1 = 2 * hp, 2 * hp + 1
                    y_pss = ypss_store.pop((n, hp))
                    yt = p_y.tile([128, 512], dt.bfloat16, tag="yt", name=f"yt{n}_{hp}")
                    ag_in = p_dram.tile([128, 512], dt.bfloat16, tag="agi",
                                        name=f"agi{n}_{hp}")
                    ag_out = p_dram.tile([256, 512], dt.bfloat16, tag="ago",
                                         name=f"ago{n}_{hp}")
                    # interleave so the gpsimd broadcasts overlap DVE work
                    recs, bcs = {}, {}
                    for h in (h0, h1):
                        rec_s = p_rec.tile([128, 512], dt.float32, tag="recs")
                        rec = p_rec.tile([128, 512], dt.float32, tag="rec")
                        nc.vector.tensor_copy(rec_s[0:1, :], y_pss[h][64:65, :])
                        nc.vector.reciprocal_approx_fast(out=rec[0:1, :], in_=rec_s[0:1, :])
                        recs[h] = rec
                        bcs[h] = p_bc.tile([128, 512], dt.float32, tag="bc",
                                           name=f"bc{n}_{h}")
                        nc.gpsimd.partition_broadcast(bcs[h][0:64, :], recs[h][0:1, :],
                                                      channels=64)
                    last = (n == 3 and hp == 3)
                    for h in (h0, h1):
                        base = (h % 2) * 64
                        nc.vector.tensor_mul(yt[base:base + 64, :], y_pss[h][0:64, :],
                                             bcs[h][0:64, :])
                        # stage each head's half as soon as it is normalized;
                        # the last norm stages via ACT's queue to dodge
                        # head-of-line AG waits on sync
                        eng = nc.scalar if last else nc.sync
                        eng.dma_start(ag_in[base:base + 64, :], yt[base:base + 64, :])

                    def do_ag(n=n, hp=hp, ag_in=ag_in, ag_out=ag_out):
                        nc.gpsimd.collective_compute(
                            "AllGather", mybir.AluOpType.bypass,
                            ins=[ag_in[:]], outs=[ag_out[:]], replica_groups=RG)
                        for half in range(2):
                            g = p_yg.tile([128, 512], dt.bfloat16, tag="yg",
                                          name=f"yg{n}_{2 * hp + half}")
                            nc.sync.dma_start(g[:], ag_out[half * 128:(half + 1) * 128, :])
                            yg_tiles.setdefault(n, []).append(g)

                    # defer the AG trigger so its gpsimd wait doesn't block the
                    # next head-pair's affine_selects; the last one is critical
                    if last:
                        flush_ags()
                        do_ag()
                    else:
                        pending_ag.append(do_ag)

                def emit_c_chunk(n, co, on_act=False):
                    flush_ags()
                    o_ps = pp_wk.tile([128, 1024], dt.float32, tag="wk")
                    for p in range(8):
                        nc.tensor.matmul(o_ps[:, 0:512], wp_sb[p][:, co * 128:(co + 1) * 128],
                                         yg_tiles[n][p][:], start=(p == 0), stop=(p == 7))
                    o_sb = p_out.tile([128, 512], dt.float32)
                    # epilogue chunks add bias on ACT (idle there); mid-step
                    # fillers use DVE to keep the exp stream unblocked
                    if on_act:
                        nc.scalar.activation(o_sb[:], o_ps[:, 0:512], AF.Identity,
                                             bias=bp_sb[:, co:co + 1])
                    else:
                        nc.vector.tensor_scalar_add(o_sb[:], o_ps[:, 0:512], bp_sb[:, co:co + 1])
                    nc.sync.dma_start(out_e[co * 128:(co + 1) * 128, n * 512:(n + 1) * 512],
                                      o_sb[:])

                def emit_filler(f):
                    if f[0] == "q":
                        emit_q_slice(f[1], f[2])
                    else:
                        emit_c_chunk(f[1], f[2])

                for step in range(5):
                    bn = step - 1
                    qn = step if step < NQ else -1
                    cn = step - 2
                    fillers = []
                    if qn >= 0:
                        fillers += [("q", qn, mq) for mq in range(4)]
                    if cn >= 0 and step < 4:
                        fillers += [("c", cn, co) for co in range(4)]
                    if bn < 0:
                        for f in fillers:
                            emit_filler(f)
                        continue
                    pairs_total = (2 * bn + 2) * 4
                    k = 0
                    fi = 0
                    for hp in range(4):
                        npair = 2 * bn + 2
                        for j in range(npair):
                            emit_scores_pair(bn, hp, j)
                            while fi < len(fillers) and fi * pairs_total < (k + 1) * len(fillers):
                                emit_filler(fillers[fi])
                                fi += 1
                            if j >= 1:
                                emit_avs_pair(bn, hp, j - 1)
                            k += 1
                        emit_avs_pair(bn, hp, npair - 1)
                        emit_norm(bn, hp)
                    while fi < len(fillers):
                        emit_filler(fillers[fi])
                        fi += 1

                # epilogue: the held-back chunk-2 projections (independent of
                # the final AllGather) fill the PE while it is in flight, then
                # chunk 3's projection. Pieces 0..5 emit piece-major; the last
                # two pieces run co-major so each co's bias-add + store
                # pipelines as soon as its group stops.
                for co in range(4):
                    emit_c_chunk(2, co, on_act=True)
                n = 3
                opsA = pp_wk.tile([128, 1024], dt.float32, tag="wk")
                opsB = pp_wk.tile([128, 1024], dt.float32, tag="wk")
                regions = [opsA[:, 0:512], opsA[:, 512:1024],
                           opsB[:, 0:512], opsB[:, 512:1024]]
                for p in range(6):
                    for co in range(4):
                        nc.tensor.matmul(regions[co], wp_sb[p][:, co * 128:(co + 1) * 128],
                                         yg_tiles[n][p][:], start=(p == 0), stop=False)
                for co in range(4):
                    for p in (6, 7):
                        nc.tensor.matmul(regions[co], wp_sb[p][:, co * 128:(co + 1) * 128],
                                         yg_tiles[n][p][:], start=False, stop=(p == 7))
                    o_sb = p_out.tile([128, 512], dt.float32)
                    nc.scalar.activation(o_sb[:], regions[co], AF.Identity,
                                         bias=bp_sb[:, co:co + 1])
                    nc.sync.dma_start(out_e[co * 128:(co + 1) * 128, n * 512:(n + 1) * 512],
                                      o_sb[:])

    nc.finalize()
    return nc


def _get_nc():
    if "nc" not in _CACHE:
        _CACHE["nc"] = _build_nc()
    return _CACHE["nc"]


def _make_in_maps(x, W_attn, b_attn, W_proj, b_proj):
    x = np.asarray(x, dtype=np.float32)
    W_attn = np.asarray(W_attn, dtype=np.float32)
    b_attn = np.asarray(b_attn, dtype=np.float32)
    W_proj = np.asarray(W_proj, dtype=np.float32)
    b_proj = np.asarray(b_proj, dtype=np.float32)

    in_maps = []
    for core in range(8):
        b, hg = core // 2, core % 2
        lo, hi = hg * CL, (hg + 1) * CL
        wq = W_attn[:, lo:hi]
        wk = W_attn[:, C + lo:C + hi]
        wv = W_attn[:, 2 * C + lo:2 * C + hi]
        bq = b_attn[lo:hi]
        bk = b_attn[C + lo:C + hi]
        bv = b_attn[2 * C + lo:2 * C + hi]
        # permuted W_proj rows: per hp, the even core's two heads then the
        # odd core's two heads (matches AllGather piece arrival order);
        # columns are this core's half of the output channels.
        wp_rows = [W_proj[128 * hp + 512 * par:128 * hp + 512 * par + 128, lo:hi]
                   for hp in range(4) for par in range(2)]
        in_maps.append({
            "xT": np.ascontiguousarray(x[b].T.astype(BF16)),
            "wkv": np.ascontiguousarray(
                np.concatenate([wk, wv], axis=1).astype(BF16)),
            "wq": np.ascontiguousarray(wq.astype(BF16)),
            "wperm": np.ascontiguousarray(
                np.concatenate(wp_rows, axis=0).astype(BF16)),
            "bqk": np.ascontiguousarray(np.concatenate([bq, bk]).reshape(8, 128).T),
            "bvr": np.ascontiguousarray(bv.reshape(1, CL).astype(BF16)),
            "bproj": np.ascontiguousarray(b_proj[lo:hi].reshape(4, 128).T),
        })
    return in_maps


def _assemble(results):
    out = np.empty((B, T, C), dtype=np.float32)
    outT = np.empty((C, T), dtype=np.float32)
    for b in range(B):
        outT[0:512, :] = results[2 * b]["out"]
        outT[512:1024, :] = results[2 * b + 1]["out"]
        out[b] = outT.T
    return out


def run(trace=False, **inputs):
    nc = _get_nc()
    in_maps = _make_in_maps(**inputs)
    kw = {}
    if trace:
        kw = dict(trace=True, trace_cores=[0])
    res = run_bass_kernel_spmd(nc, in_maps, list(range(8)), **kw)
    return _assemble(res.results), res


def kernel(**inputs) -> np.ndarray:
    out, _ = run(trace=False, **inputs)
    return out
